# revision 4
# baseline (speedup 1.0000x reference)
"""DisentangledGNN Trainium2 kernel (8 NeuronCores, SPMD).

Strategy: target-bucketed node sharding. Each core owns n/8 consecutive
nodes and every edge whose target lands in that range. Per core:
  P0: pca matmul + leaky_relu + grouped l2norm for its own node slice
  P1: AllGather of the normalized features (bf16, padded to 192 cols)
  P2: one-time gather z = H[src] for its edges (indirect DMA)
  P3: 3 routing iterations, chunk-fused: for each 128-node chunk, edges
      are processed as 128-edge tiles; per-edge "gather u[trg]" and the
      segment-sum are one-hot matmuls on the tensor engine (S matrices
      built once per chunk via iota/is_equal); softmax over the 10
      factors is exp/sum on free axis (|s|<=1 so no max subtraction).
  P4: (fused in last iteration) leaky_relu + classifier matmul.
No inter-core communication during routing: a chunk's new u depends
only on that chunk's old u and the iteration-fixed z.
"""

import numpy as np
import ml_dtypes

import concourse.bass as bass
import concourse.mybir as mybir
import concourse.tile as tile
from concourse.masks import make_identity
from concourse.bass_utils import run_bass_kernel_spmd

F32 = mybir.dt.float32
BF16 = mybir.dt.bfloat16
I32 = mybir.dt.int32
I16 = mybir.dt.int16

K = 10
SLOPE = 0.01
NITER = 3
P = 128


def _split_multiwaits(nc):
    # This walrus accepts at most 1 sync wait per instruction (2 for
    # EventSemaphore ops); split extras onto preceding same-engine NOPs.
    n = [0]
    for fn in nc.m.functions:
        for blk in fn.blocks:
            newinsts = []
            changed = False
            for ins in blk.instructions:
                si = ins.sync_info
                cap = 2 if "EventSem" in type(ins).__name__ else 1
                if si is not None and len(si.on_wait) > cap:
                    waits = list(si.on_wait)
                    for w in waits[cap:]:
                        n[0] += 1
                        nop = mybir.InstNoOp(name=f"{ins.name}-ws{n[0]}", ins=[], outs=[])
                        nop.engine = ins.engine
                        nop.sync_info = mybir.SyncInfo(on_wait=[w], on_update=[])
                        newinsts.append(nop)
                    si.on_wait = waits[:cap]
                    ins.sync_info = si
                    changed = True
                newinsts.append(ins)
            if changed:
                blk.instructions = newinsts


def _host_prep(x, edge_index, n_cores):
    """Bucket edges by target core, chunk them by 128-node blocks,
    equalize per-chunk tile counts across cores, build per-core arrays."""
    n = x.shape[0]
    npc = n // n_cores            # nodes per core
    nchunks = (npc + P - 1) // P  # 128-node chunks per core
    src = np.asarray(edge_index[0], np.int64)
    trg = np.asarray(edge_index[1], np.int64)

    core_of = trg // npc
    ltrg = trg - core_of * npc

    # sort edges by (core, ltrg) once
    order = np.lexsort((ltrg, core_of))
    src_s, ltrg_s, core_s = src[order], ltrg[order], core_of[order]

    # per (core, chunk) counts
    chunk_s = ltrg_s // P
    counts = np.zeros((n_cores, nchunks), np.int64)
    np.add.at(counts, (core_s, chunk_s), 1)
    nt = np.maximum(1, (counts.max(axis=0) + P - 1) // P)  # tiles per chunk (shared)
    T = int(nt.sum())

    src_arr = np.zeros((n_cores, T * P), np.int32)
    lloc_arr = np.full((n_cores, T * P), 255, np.int16)  # 255 = dummy, never matches
    tile_of_chunk = np.concatenate([[0], np.cumsum(nt)]).astype(np.int64)

    core_starts = np.searchsorted(core_s, np.arange(n_cores + 1))
    for c in range(n_cores):
        cs, ce = core_starts[c], core_starts[c + 1]
        chunk_c = chunk_s[cs:ce]
        starts = np.searchsorted(chunk_c, np.arange(nchunks + 1))
        for j in range(nchunks):
            e0, e1 = cs + starts[j], cs + starts[j + 1]
            base = int(tile_of_chunk[j]) * P
            cnt = e1 - e0
            src_arr[c, base:base + cnt] = src_s[e0:e1]
            lloc_arr[c, base:base + cnt] = (ltrg_s[e0:e1] - j * P).astype(np.int16)

    # device wants [128, T] partition-major: edge t*128+p -> [p, t]
    src_dev = src_arr.reshape(n_cores, T, P).transpose(0, 2, 1).copy()
    lloc_dev = lloc_arr.reshape(n_cores, T, P).transpose(0, 2, 1).copy()
    return nt, T, src_dev, lloc_dev, npc, nchunks


def build_program(nfeat, d, nclass, npc, nchunks, nt, T, n_cores, n_nodes):
    dd = d // K
    kf = nfeat  # contraction for pca, padded to x128 on host
    kf_pad = ((nfeat + P - 1) // P) * P
    npc_pad = nchunks * P
    DPAD = 192  # gather row padded to 384B (bf16)

    nc = bass.Bass(num_devices=n_cores)

    xT_t = nc.dram_tensor("xT", [kf_pad, npc_pad], F32, kind="ExternalInput")
    w_t = nc.dram_tensor("pca_w", [kf_pad, d], F32, kind="ExternalInput")
    brep_t = nc.dram_tensor("pca_b_rep", [P, d], F32, kind="ExternalInput")
    cw_t = nc.dram_tensor("clf_w", [d, nclass], F32, kind="ExternalInput")
    cbrep_t = nc.dram_tensor("clf_b_rep", [P, nclass], F32, kind="ExternalInput")
    src_t = nc.dram_tensor("src", [P, T], I32, kind="ExternalInput")
    lloc_t = nc.dram_tensor("lloc", [P, T], I16, kind="ExternalInput")
    y_t = nc.dram_tensor("y", [npc_pad, nclass], F32, kind="ExternalOutput")
    Hp = nc.dram_tensor("Hp", [n_nodes, DPAD], BF16, kind="Internal")

    with tile.TileContext(nc) as tc:
        with (
            tc.tile_pool(name="persist", bufs=1) as pp,
            tc.tile_pool(name="dram", bufs=1, space="DRAM") as dp,
            tc.tile_pool(name="sb", bufs=3) as sb,
            tc.tile_pool(name="sb1", bufs=2) as sb1,
            tc.tile_pool(name="schunk", bufs=2) as sc,
            tc.tile_pool(name="ps", bufs=3, space="PSUM") as psp,
            tc.tile_pool(name="psu", bufs=2, space="PSUM") as psu,
        ):
            # ---------------- constants / persistent state ----------------
            iota_rep = pp.tile([P, P], I16)
            nc.gpsimd.iota(iota_rep[:], pattern=[[1, P]], base=0, channel_multiplier=0)
            ident = pp.tile([P, P], BF16)
            make_identity(nc, ident[:])
            idf = pp.tile([P, P], F32)
            make_identity(nc, idf[:])

            nkt0 = kf_pad // P
            w_sb = pp.tile([P, nkt0 * d], F32)  # pca_w K-tiles side by side
            nc.sync.dma_start(
                out=w_sb[:].rearrange("p (a d) -> p a d", d=d),
                in_=w_t[:].rearrange("(a p) d -> p a d", p=P),
            )
            brep = pp.tile([P, d], F32)
            nc.sync.dma_start(out=brep[:], in_=brep_t[:])
            cw_sb = pp.tile([P, 2 * nclass], F32)  # clf_w K-tiles: [0:128], [128:160]
            nc.sync.dma_start(out=cw_sb[:, :nclass], in_=cw_t[:P, :])
            nc.sync.dma_start(out=cw_sb[: d - P, nclass:], in_=cw_t[P:, :])
            cbrep = pp.tile([P, nclass], F32)
            nc.sync.dma_start(out=cbrep[:], in_=cbrep_t[:])

            hn = pp.tile([P, nchunks * d], F32)   # normalized features, own nodes
            nc.vector.memset(hn[:], 0.0)

            # bounce buffers for allgather
            ag_in = dp.tile([npc, DPAD], BF16)
            zdr = dp.tile([P, T * d], BF16)       # z in [128, T, d] partition-major

            # ---------------- P0: pca + lrelu + l2norm --------------------
            nkt = kf_pad // P
            for m in range(nchunks):
                xt = sb.tile([P, nkt * P], F32, tag="xt")
                nc.sync.dma_start(
                    out=xt[:].rearrange("p (a q) -> p a q", q=P),
                    in_=xT_t[:, m * P:(m + 1) * P].rearrange("(a p) q -> p a q", p=P),
                )
                h_ps = psp.tile([P, d], F32, space="PSUM", tag="big")
                for a in range(nkt):
                    nc.tensor.matmul(
                        out=h_ps[:],
                        lhsT=xt[:, a * P:(a + 1) * P],
                        rhs=w_sb[:, a * d:(a + 1) * d],
                        start=(a == 0),
                        stop=(a == nkt - 1),
                    )
                h = sb.tile([P, d], F32, tag="h_sb")
                nc.vector.tensor_add(out=h[:], in0=h_ps[:], in1=brep[:])
                hs = sb.tile([P, d], F32, tag="hs")
                nc.vector.tensor_scalar_mul(out=hs[:], in0=h[:], scalar1=SLOPE)
                nc.vector.tensor_tensor(out=h[:], in0=h[:], in1=hs[:], op=mybir.AluOpType.max)
                # grouped l2 norm
                sq = sb.tile([P, d], F32, tag="sq")
                nc.vector.tensor_mul(out=sq[:], in0=h[:], in1=h[:])
                ss = sb.tile([P, K], F32, tag="ss")
                nc.vector.reduce_sum(
                    out=ss[:], in_=sq[:].rearrange("p (k e) -> p k e", k=K),
                    axis=mybir.AxisListType.X,
                )
                nrm = sb.tile([P, K], F32, tag="nrm")
                nc.scalar.activation(out=nrm[:], in_=ss[:], func=mybir.ActivationFunctionType.Sqrt)
                nc.vector.tensor_scalar_max(out=nrm[:], in0=nrm[:], scalar1=1e-12)
                rr = sb.tile([P, K], F32, tag="rr")
                nc.vector.reciprocal(out=rr[:], in_=nrm[:])
                nc.vector.tensor_tensor(
                    out=hn[:, m * d:(m + 1) * d].rearrange("p (k e) -> p k e", k=K),
                    in0=h[:].rearrange("p (k e) -> p k e", k=K),
                    in1=rr[:].unsqueeze(2).to_broadcast([P, K, dd]),
                    op=mybir.AluOpType.mult,
                )
                # bf16 padded copy for allgather
                hb = sb.tile([P, DPAD], BF16, tag="hb")
                nc.vector.memset(hb[:], 0.0)
                nc.vector.tensor_copy(out=hb[:, :d], in_=hn[:, m * d:(m + 1) * d])
                rows = min(P, npc - m * P)
                nc.sync.dma_start(out=ag_in[m * P:m * P + rows, :], in_=hb[:rows, :])

            # ---------------- P1: allgather -------------------------------
            nc.gpsimd.collective_compute(
                "AllGather",
                mybir.AluOpType.bypass,
                replica_groups=[list(range(n_cores))],
                ins=[ag_in[:]],
                outs=[Hp.ap()],
            )

            # ---------------- P2: z gather --------------------------------
            src_sb = pp.tile([P, T], I32)
            nc.sync.dma_start(out=src_sb[:], in_=src_t[:])
            for t in range(T):
                g = sb.tile([P, DPAD], BF16, tag="zg")
                nc.gpsimd.indirect_dma_start(
                    out=g[:],
                    out_offset=None,
                    in_=Hp.ap(),
                    in_offset=bass.IndirectOffsetOnAxis(ap=src_sb[:, t:t + 1], axis=0),
                )
                nc.sync.dma_start(
                    out=zdr[:].rearrange("p (t e) -> p t e", t=T)[:, t, :],
                    in_=g[:, :d],
                )

            # ---------------- P3: routing ---------------------------------
            lloc_all = pp.tile([P, T], I16)
            nc.sync.dma_start(out=lloc_all[:], in_=lloc_t[:])

            GT = 8  # tiles per DVE batch group
            for j in range(nchunks):
                t0, t1 = int(np.sum(nt[:j])), int(np.sum(nt[:j + 1]))
                ntj = t1 - t0
                # chunk-resident data
                zch = sc.tile([P, ntj * d], BF16, tag="zch")
                nc.sync.dma_start(out=zch[:], in_=zdr[:, t0 * d:t1 * d])
                S_sb = sc.tile([P, ntj * P], BF16, tag="S")
                ST_sb = sc.tile([P, ntj * P], BF16, tag="ST")
                for t in range(ntj):
                    nc.vector.tensor_tensor(
                        out=S_sb[:, t * P:(t + 1) * P],
                        in0=lloc_all[:, t0 + t:t0 + t + 1].to_broadcast([P, P]),
                        in1=iota_rep[:],
                        op=mybir.AluOpType.is_equal,
                    )
                # transposes in batches of 4 per PSUM tile
                for b0 in range(0, ntj, 4):
                    bn = min(4, ntj - b0)
                    tr_ps = psp.tile([P, 4 * P], BF16, space="PSUM", tag="big")
                    for t in range(bn):
                        nc.tensor.transpose(
                            out=tr_ps[:, t * P:(t + 1) * P],
                            in_=S_sb[:, (b0 + t) * P:(b0 + t + 1) * P],
                            identity=ident[:],
                        )
                    nc.scalar.copy(
                        out=ST_sb[:, b0 * P:(b0 + bn) * P], in_=tr_ps[:, :bn * P]
                    )

                u_j = sc.tile([P, d], BF16, tag="uj")
                nc.vector.tensor_copy(out=u_j[:], in_=hn[:, j * d:(j + 1) * d])

                for it in range(NITER):
                    seg_ps = psu.tile([P, d], F32, space="PSUM", tag="seg")
                    for g0 in range(0, ntj, GT):
                        gn = min(GT, ntj - g0)
                        ut_bf = sb1.tile([P, GT * d], BF16, tag="utbf")
                        for b0 in range(g0, g0 + gn, 3):
                            bn = min(3, g0 + gn - b0)
                            ut_ps = psp.tile([P, 3 * d], F32, space="PSUM", tag="big")
                            for t in range(bn):
                                nc.tensor.matmul(
                                    out=ut_ps[:, t * d:(t + 1) * d],
                                    lhsT=ST_sb[:, (b0 + t) * P:(b0 + t + 1) * P],
                                    rhs=u_j[:],
                                    start=True, stop=True,
                                )
                            nc.vector.tensor_copy(
                                out=ut_bf[:, (b0 - g0) * d:(b0 - g0 + bn) * d],
                                in_=ut_ps[:, :bn * d],
                            )
                        prod = sb1.tile([P, GT * d], BF16, tag="prod")
                        nc.vector.tensor_mul(
                            out=prod[:, :gn * d],
                            in0=zch[:, g0 * d:(g0 + gn) * d],
                            in1=ut_bf[:, :gn * d],
                        )
                        s_f = sb1.tile([P, GT * K], F32, tag="sf")
                        nc.vector.reduce_sum(
                            out=s_f[:, :gn * K],
                            in_=prod[:, :gn * d].rearrange("p (a e) -> p a e", e=dd),
                            axis=mybir.AxisListType.X,
                        )
                        e_f = sb1.tile([P, GT * K], F32, tag="ef")
                        nc.scalar.activation(
                            out=e_f[:, :gn * K], in_=s_f[:, :gn * K],
                            func=mybir.ActivationFunctionType.Exp,
                        )
                        q_f = sb1.tile([P, GT], F32, tag="qf")
                        nc.vector.reduce_sum(
                            out=q_f[:, :gn],
                            in_=e_f[:, :gn * K].rearrange("p (a k) -> p a k", k=K),
                            axis=mybir.AxisListType.X,
                        )
                        r_f = sb1.tile([P, GT], F32, tag="rf")
                        nc.vector.reciprocal(out=r_f[:, :gn], in_=q_f[:, :gn])
                        pe_f = sb1.tile([P, GT * K], BF16, tag="pef")
                        nc.vector.tensor_tensor(
                            out=pe_f[:, :gn * K].rearrange("p (a k) -> p a k", k=K),
                            in0=e_f[:, :gn * K].rearrange("p (a k) -> p a k", k=K),
                            in1=r_f[:, :gn].unsqueeze(2).to_broadcast([P, gn, K]),
                            op=mybir.AluOpType.mult,
                        )
                        msg = sb1.tile([P, GT * d], BF16, tag="msg")
                        nc.vector.tensor_tensor(
                            out=msg[:, :gn * d].rearrange("p (a e) -> p a e", e=dd),
                            in0=zch[:, g0 * d:(g0 + gn) * d].rearrange("p (a e) -> p a e", e=dd),
                            in1=pe_f[:, :gn * K].unsqueeze(2).to_broadcast([P, gn * K, dd]),
                            op=mybir.AluOpType.mult,
                        )
                        for t in range(gn):
                            nc.tensor.matmul(
                                out=seg_ps[:],
                                lhsT=S_sb[:, (g0 + t) * P:(g0 + t + 1) * P],
                                rhs=msg[:, t * d:(t + 1) * d],
                                start=(g0 + t == 0),
                                stop=(g0 + t == ntj - 1),
                            )
                    # chunk epilogue: u = l2norm(seg + hn)
                    tt = sc.tile([P, d], F32, tag="tt")
                    nc.vector.tensor_add(out=tt[:], in0=seg_ps[:], in1=hn[:, j * d:(j + 1) * d])
                    sq2 = sc.tile([P, d], F32, tag="sq2")
                    nc.vector.tensor_mul(out=sq2[:], in0=tt[:], in1=tt[:])
                    ss2 = sc.tile([P, K], F32, tag="ss2")
                    nc.vector.reduce_sum(
                        out=ss2[:], in_=sq2[:].rearrange("p (k e) -> p k e", k=K),
                        axis=mybir.AxisListType.X,
                    )
                    nr2 = sc.tile([P, K], F32, tag="nr2")
                    nc.scalar.activation(out=nr2[:], in_=ss2[:], func=mybir.ActivationFunctionType.Sqrt)
                    nc.vector.tensor_scalar_max(out=nr2[:], in0=nr2[:], scalar1=1e-12)
                    rr2 = sc.tile([P, K], F32, tag="rr2")
                    nc.vector.reciprocal(out=rr2[:], in_=nr2[:])
                    if it < NITER - 1:
                        nc.vector.tensor_tensor(
                            out=u_j[:].rearrange("p (k e) -> p k e", k=K),
                            in0=tt[:].rearrange("p (k e) -> p k e", k=K),
                            in1=rr2[:].unsqueeze(2).to_broadcast([P, K, dd]),
                            op=mybir.AluOpType.mult,
                        )
                    else:
                        # final: u (f32) -> lrelu -> clf matmul -> y
                        uf = sc.tile([P, d], F32, tag="uf")
                        nc.vector.tensor_tensor(
                            out=uf[:].rearrange("p (k e) -> p k e", k=K),
                            in0=tt[:].rearrange("p (k e) -> p k e", k=K),
                            in1=rr2[:].unsqueeze(2).to_broadcast([P, K, dd]),
                            op=mybir.AluOpType.mult,
                        )
                        us = sc.tile([P, d], F32, tag="us")
                        nc.vector.tensor_scalar_mul(out=us[:], in0=uf[:], scalar1=SLOPE)
                        nc.vector.tensor_tensor(out=uf[:], in0=uf[:], in1=us[:], op=mybir.AluOpType.max)
                        # transpose uf -> [d, nodes] K-tiles
                        uT_ps = psp.tile([P, 2 * P], F32, space="PSUM", tag="big")
                        nc.tensor.transpose(out=uT_ps[:, :P], in_=uf[:, :P], identity=idf[:])
                        nc.tensor.transpose(
                            out=uT_ps[: d - P, P:2 * P], in_=uf[:, P:d], identity=idf[:]
                        )
                        uT = sc.tile([P, 2 * P], F32, tag="uTs")
                        nc.vector.tensor_copy(out=uT[:, :P], in_=uT_ps[:, :P])
                        nc.vector.tensor_copy(out=uT[: d - P, P:], in_=uT_ps[: d - P, P:])
                        y_ps = psp.tile([P, nclass], F32, space="PSUM", tag="big")
                        nc.tensor.matmul(
                            out=y_ps[:], lhsT=uT[:, :P], rhs=cw_sb[:, :nclass],
                            start=True, stop=False,
                        )
                        nc.tensor.matmul(
                            out=y_ps[:], lhsT=uT[: d - P, P:], rhs=cw_sb[: d - P, nclass:],
                            start=False, stop=True,
                        )
                        y_sb = sc.tile([P, nclass], F32, tag="ysb")
                        nc.vector.tensor_add(out=y_sb[:], in0=y_ps[:], in1=cbrep[:])
                        nc.sync.dma_start(out=y_t[j * P:(j + 1) * P, :], in_=y_sb[:])
    return nc


_CACHE = {}
TRACE = False
LAST_RESULTS = None


def kernel(x, edge_index, pca_w, pca_b, clf_w, clf_b, n_cores=8, _sim=False):
    x = np.asarray(x, np.float32)
    edge_index = np.asarray(edge_index)
    idx_dtype = edge_index.dtype
    pca_w = np.asarray(pca_w, np.float32)
    pca_b = np.asarray(pca_b, np.float32)
    clf_w = np.asarray(clf_w, np.float32)
    clf_b = np.asarray(clf_b, np.float32)

    n, nfeat = x.shape
    d = pca_w.shape[1]
    nclass = clf_w.shape[1]

    nt, T, src_dev, lloc_dev, npc, nchunks = _host_prep(x, edge_index, n_cores)

    key = (n, nfeat, d, nclass, tuple(nt.tolist()))
    if key not in _CACHE:
        _CACHE[key] = build_program(nfeat, d, nclass, npc, nchunks, nt, T, n_cores, n)
        if not _sim:
            _split_multiwaits(_CACHE[key])
    nc = _CACHE[key]

    kf_pad = ((nfeat + P - 1) // P) * P
    npc_pad = nchunks * P
    w_pad = np.zeros((kf_pad, d), np.float32)
    w_pad[:nfeat] = pca_w
    brep = np.broadcast_to(pca_b, (P, d)).copy()
    cbrep = np.broadcast_to(clf_b, (P, nclass)).copy()

    in_maps = []
    for c in range(n_cores):
        xc = x[c * npc:(c + 1) * npc]
        xT = np.zeros((kf_pad, npc_pad), np.float32)
        xT[:nfeat, :npc] = xc.T
        in_maps.append({
            "xT": xT,
            "pca_w": w_pad,
            "pca_b_rep": brep,
            "clf_w": clf_w,
            "clf_b_rep": cbrep,
            "src": src_dev[c],
            "lloc": lloc_dev[c],
        })

    if _sim:
        from concourse.bass_interp import CoreSim
        assert n_cores == 1
        sim = CoreSim(nc)
        for kk, vv in in_maps[0].items():
            sim.tensor(kk)[:] = vv
        sim.simulate()
        return np.asarray(sim.tensor("y"))[:npc].astype(np.float32)
    global LAST_RESULTS
    res = run_bass_kernel_spmd(
        nc, in_maps, core_ids=list(range(n_cores)), trace=TRACE
    )
    LAST_RESULTS = res
    y = np.concatenate([res.results[c]["y"][:npc] for c in range(n_cores)], axis=0)
    return y.astype(np.float32)


if __name__ == "__main__":
    import pickle, time
    with open("/tmp/ref_inputs.pkl", "rb") as f:
        inputs = pickle.load(f)
    t0 = time.time()
    y = kernel(**inputs)
    print("kernel() wall time", time.time() - t0)
    np.save("/tmp/kernel_out.npy", y)


# revision 5
# speedup vs baseline: 1.2244x; 1.2244x over previous
"""DisentangledGNN Trainium2 kernel (8 NeuronCores, SPMD).

Strategy: target-bucketed node sharding. Each core owns n/8 consecutive
nodes and every edge whose target lands in that range. Per core:
  P0: pca matmul + leaky_relu + grouped l2norm for its own node slice
  P1: AllGather of the normalized features (bf16, padded to 192 cols)
  P2: one-time gather z = H[src] for its edges (indirect DMA)
  P3: 3 routing iterations, chunk-fused: for each 128-node chunk, edges
      are processed as 128-edge tiles; per-edge "gather u[trg]" and the
      segment-sum are one-hot matmuls on the tensor engine (S matrices
      built once per chunk via iota/is_equal); softmax over the 10
      factors is exp/sum on free axis (|s|<=1 so no max subtraction).
  P4: (fused in last iteration) leaky_relu + classifier matmul.
No inter-core communication during routing: a chunk's new u depends
only on that chunk's old u and the iteration-fixed z.
"""

import numpy as np
import ml_dtypes

import concourse.bass as bass
import concourse.mybir as mybir
import concourse.tile as tile
from concourse.masks import make_identity
from concourse.bass_utils import run_bass_kernel_spmd

F32 = mybir.dt.float32
BF16 = mybir.dt.bfloat16
I32 = mybir.dt.int32
I16 = mybir.dt.int16

K = 10
SLOPE = 0.01
NITER = 3
P = 128


def _split_multiwaits(nc):
    # This walrus accepts at most 1 sync wait per instruction (2 for
    # EventSemaphore ops); split extras onto preceding same-engine NOPs.
    n = [0]
    for fn in nc.m.functions:
        for blk in fn.blocks:
            newinsts = []
            changed = False
            for ins in blk.instructions:
                si = ins.sync_info
                cap = 2 if "EventSem" in type(ins).__name__ else 1
                if si is not None and len(si.on_wait) > cap:
                    waits = list(si.on_wait)
                    for w in waits[cap:]:
                        n[0] += 1
                        nop = mybir.InstNoOp(name=f"{ins.name}-ws{n[0]}", ins=[], outs=[])
                        nop.engine = ins.engine
                        nop.sync_info = mybir.SyncInfo(on_wait=[w], on_update=[])
                        newinsts.append(nop)
                    si.on_wait = waits[:cap]
                    ins.sync_info = si
                    changed = True
                newinsts.append(ins)
            if changed:
                blk.instructions = newinsts


def _host_prep(x, edge_index, n_cores):
    """Bucket edges by target core, chunk them by 128-node blocks,
    equalize per-chunk tile counts across cores, build per-core arrays."""
    n = x.shape[0]
    npc = n // n_cores            # nodes per core
    nchunks = (npc + P - 1) // P  # 128-node chunks per core
    src = np.asarray(edge_index[0], np.int64)
    trg = np.asarray(edge_index[1], np.int64)

    core_of = trg // npc
    ltrg = trg - core_of * npc

    # sort edges by (core, ltrg) once
    order = np.lexsort((ltrg, core_of))
    src_s, ltrg_s, core_s = src[order], ltrg[order], core_of[order]

    # per (core, chunk) counts
    chunk_s = ltrg_s // P
    counts = np.zeros((n_cores, nchunks), np.int64)
    np.add.at(counts, (core_s, chunk_s), 1)
    nt = np.maximum(1, (counts.max(axis=0) + P - 1) // P)  # tiles per chunk (shared)
    T = int(nt.sum())

    src_arr = np.zeros((n_cores, T * P), np.int32)
    lloc_arr = np.full((n_cores, T * P), 255, np.int16)  # 255 = dummy, never matches
    tile_of_chunk = np.concatenate([[0], np.cumsum(nt)]).astype(np.int64)

    core_starts = np.searchsorted(core_s, np.arange(n_cores + 1))
    for c in range(n_cores):
        cs, ce = core_starts[c], core_starts[c + 1]
        chunk_c = chunk_s[cs:ce]
        starts = np.searchsorted(chunk_c, np.arange(nchunks + 1))
        for j in range(nchunks):
            e0, e1 = cs + starts[j], cs + starts[j + 1]
            base = int(tile_of_chunk[j]) * P
            cnt = e1 - e0
            src_arr[c, base:base + cnt] = src_s[e0:e1]
            lloc_arr[c, base:base + cnt] = (ltrg_s[e0:e1] - j * P).astype(np.int16)

    # device wants [128, T] partition-major: edge t*128+p -> [p, t]
    src_dev = src_arr.reshape(n_cores, T, P).transpose(0, 2, 1).copy()
    lloc_dev = lloc_arr.reshape(n_cores, T, P).transpose(0, 2, 1).copy()
    return nt, T, src_dev, lloc_dev, npc, nchunks


def build_program(nfeat, d, nclass, npc, nchunks, nt, T, n_cores, n_nodes):
    dd = d // K
    kf = nfeat  # contraction for pca, padded to x128 on host
    kf_pad = ((nfeat + P - 1) // P) * P
    npc_pad = nchunks * P
    DPAD = d

    nc = bass.Bass(num_devices=n_cores)

    xT_t = nc.dram_tensor("xT", [kf_pad, npc_pad], F32, kind="ExternalInput")
    w_t = nc.dram_tensor("pca_w", [kf_pad, d], F32, kind="ExternalInput")
    brep_t = nc.dram_tensor("pca_b_rep", [P, d], F32, kind="ExternalInput")
    cw_t = nc.dram_tensor("clf_w", [d, nclass], F32, kind="ExternalInput")
    cbrep_t = nc.dram_tensor("clf_b_rep", [P, nclass], F32, kind="ExternalInput")
    src_t = nc.dram_tensor("src", [P, T], I32, kind="ExternalInput")
    lloc_t = nc.dram_tensor("lloc", [P, T], I16, kind="ExternalInput")
    y_t = nc.dram_tensor("y", [npc_pad, nclass], F32, kind="ExternalOutput")
    Hp = nc.dram_tensor("Hp", [n_nodes, DPAD], BF16, kind="Internal")

    with tile.TileContext(nc) as tc:
        with (
            tc.tile_pool(name="persist", bufs=1) as pp,
            tc.tile_pool(name="dram", bufs=1, space="DRAM") as dp,
            tc.tile_pool(name="sb", bufs=3) as sb,
            tc.tile_pool(name="sb1", bufs=2) as sb1,
            tc.tile_pool(name="schunk", bufs=2) as sc,
            tc.tile_pool(name="ps", bufs=3, space="PSUM") as psp,
            tc.tile_pool(name="psu", bufs=2, space="PSUM") as psu,
        ):
            # ---------------- constants / persistent state ----------------
            iota_rep = pp.tile([P, P], I16)
            nc.gpsimd.iota(iota_rep[:], pattern=[[1, P]], base=0, channel_multiplier=0)
            ident = pp.tile([P, P], BF16)
            make_identity(nc, ident[:])
            idf = pp.tile([P, P], F32)
            make_identity(nc, idf[:])

            nkt0 = kf_pad // P
            w_sb = pp.tile([P, nkt0 * d], F32)  # pca_w K-tiles side by side
            nc.sync.dma_start(
                out=w_sb[:].rearrange("p (a d) -> p a d", d=d),
                in_=w_t[:].rearrange("(a p) d -> p a d", p=P),
            )
            brep = pp.tile([P, d], F32)
            nc.sync.dma_start(out=brep[:], in_=brep_t[:])
            cw_sb = pp.tile([P, 2 * nclass], F32)  # clf_w K-tiles: [0:128], [128:160]
            nc.sync.dma_start(out=cw_sb[:, :nclass], in_=cw_t[:P, :])
            nc.sync.dma_start(out=cw_sb[: d - P, nclass:], in_=cw_t[P:, :])
            cbrep = pp.tile([P, nclass], F32)
            nc.sync.dma_start(out=cbrep[:], in_=cbrep_t[:])

            hn = pp.tile([P, nchunks * d], F32)   # normalized features, own nodes
            nc.vector.memset(hn[:], 0.0)

            # bounce buffers for allgather
            ag_in = dp.tile([npc, DPAD], BF16)

            # ---------------- P0: pca + lrelu + l2norm --------------------
            nkt = kf_pad // P
            for m in range(nchunks):
                xt = sb.tile([P, nkt * P], F32, tag="xt")
                nc.sync.dma_start(
                    out=xt[:].rearrange("p (a q) -> p a q", q=P),
                    in_=xT_t[:, m * P:(m + 1) * P].rearrange("(a p) q -> p a q", p=P),
                )
                h_ps = psp.tile([P, d], F32, space="PSUM", tag="big")
                for a in range(nkt):
                    nc.tensor.matmul(
                        out=h_ps[:],
                        lhsT=xt[:, a * P:(a + 1) * P],
                        rhs=w_sb[:, a * d:(a + 1) * d],
                        start=(a == 0),
                        stop=(a == nkt - 1),
                    )
                h = sb.tile([P, d], F32, tag="h_sb")
                nc.vector.tensor_add(out=h[:], in0=h_ps[:], in1=brep[:])
                hs = sb.tile([P, d], F32, tag="hs")
                nc.vector.tensor_scalar_mul(out=hs[:], in0=h[:], scalar1=SLOPE)
                nc.vector.tensor_tensor(out=h[:], in0=h[:], in1=hs[:], op=mybir.AluOpType.max)
                # grouped l2 norm
                sq = sb.tile([P, d], F32, tag="sq")
                nc.vector.tensor_mul(out=sq[:], in0=h[:], in1=h[:])
                ss = sb.tile([P, K], F32, tag="ss")
                nc.vector.reduce_sum(
                    out=ss[:], in_=sq[:].rearrange("p (k e) -> p k e", k=K),
                    axis=mybir.AxisListType.X,
                )
                nrm = sb.tile([P, K], F32, tag="nrm")
                nc.scalar.activation(out=nrm[:], in_=ss[:], func=mybir.ActivationFunctionType.Sqrt)
                nc.vector.tensor_scalar_max(out=nrm[:], in0=nrm[:], scalar1=1e-12)
                rr = sb.tile([P, K], F32, tag="rr")
                nc.vector.reciprocal(out=rr[:], in_=nrm[:])
                nc.vector.tensor_tensor(
                    out=hn[:, m * d:(m + 1) * d].rearrange("p (k e) -> p k e", k=K),
                    in0=h[:].rearrange("p (k e) -> p k e", k=K),
                    in1=rr[:].unsqueeze(2).to_broadcast([P, K, dd]),
                    op=mybir.AluOpType.mult,
                )
                # bf16 padded copy for allgather
                hb = sb.tile([P, DPAD], BF16, tag="hb")
                nc.vector.tensor_copy(out=hb[:], in_=hn[:, m * d:(m + 1) * d])
                rows = min(P, npc - m * P)
                nc.sync.dma_start(out=ag_in[m * P:m * P + rows, :], in_=hb[:rows, :])

            # ---------------- P1: allgather -------------------------------
            nc.gpsimd.collective_compute(
                "AllGather",
                mybir.AluOpType.bypass,
                replica_groups=[list(range(n_cores))],
                ins=[ag_in[:]],
                outs=[Hp.ap()],
            )

            # ---------------- P2: (z gathered per chunk in P3) -------------
            src_sb = pp.tile([P, T], I32)
            nc.sync.dma_start(out=src_sb[:], in_=src_t[:])

            # ---------------- P3: routing ---------------------------------
            lloc_all = pp.tile([P, T], I16)
            nc.sync.dma_start(out=lloc_all[:], in_=lloc_t[:])

            GT = 8  # tiles per DVE batch group
            for j in range(nchunks):
                t0, t1 = int(np.sum(nt[:j])), int(np.sum(nt[:j + 1]))
                ntj = t1 - t0
                # chunk-resident data
                zch = sc.tile([P, ntj * d], BF16, tag="zch")
                for t in range(ntj):
                    nc.gpsimd.indirect_dma_start(
                        out=zch[:, t * d:(t + 1) * d],
                        out_offset=None,
                        in_=Hp.ap(),
                        in_offset=bass.IndirectOffsetOnAxis(
                            ap=src_sb[:, t0 + t:t0 + t + 1], axis=0
                        ),
                    )
                S_sb = sc.tile([P, ntj * P], BF16, tag="S")
                ST_sb = sc.tile([P, ntj * P], BF16, tag="ST")
                for t in range(ntj):
                    nc.vector.tensor_tensor(
                        out=S_sb[:, t * P:(t + 1) * P],
                        in0=lloc_all[:, t0 + t:t0 + t + 1].to_broadcast([P, P]),
                        in1=iota_rep[:],
                        op=mybir.AluOpType.is_equal,
                    )
                # transposes in batches of 4 per PSUM tile
                for b0 in range(0, ntj, 4):
                    bn = min(4, ntj - b0)
                    tr_ps = psp.tile([P, 4 * P], BF16, space="PSUM", tag="big")
                    for t in range(bn):
                        nc.tensor.transpose(
                            out=tr_ps[:, t * P:(t + 1) * P],
                            in_=S_sb[:, (b0 + t) * P:(b0 + t + 1) * P],
                            identity=ident[:],
                        )
                    nc.scalar.copy(
                        out=ST_sb[:, b0 * P:(b0 + bn) * P], in_=tr_ps[:, :bn * P]
                    )

                u_j = sc.tile([P, d], BF16, tag="uj")
                nc.vector.tensor_copy(out=u_j[:], in_=hn[:, j * d:(j + 1) * d])

                for it in range(NITER):
                    seg_ps = psu.tile([P, d], F32, space="PSUM", tag="seg")
                    for g0 in range(0, ntj, GT):
                        gn = min(GT, ntj - g0)
                        ut_bf = sb1.tile([P, GT * d], BF16, tag="utbf")
                        for b0 in range(g0, g0 + gn, 3):
                            bn = min(3, g0 + gn - b0)
                            ut_ps = psp.tile([P, 3 * d], F32, space="PSUM", tag="big")
                            for t in range(bn):
                                nc.tensor.matmul(
                                    out=ut_ps[:, t * d:(t + 1) * d],
                                    lhsT=ST_sb[:, (b0 + t) * P:(b0 + t + 1) * P],
                                    rhs=u_j[:],
                                    start=True, stop=True,
                                )
                            nc.scalar.copy(
                                out=ut_bf[:, (b0 - g0) * d:(b0 - g0 + bn) * d],
                                in_=ut_ps[:, :bn * d],
                            )
                        prod = sb1.tile([P, GT * d], BF16, tag="prod")
                        nc.vector.tensor_mul(
                            out=prod[:, :gn * d],
                            in0=zch[:, g0 * d:(g0 + gn) * d],
                            in1=ut_bf[:, :gn * d],
                        )
                        s_f = sb1.tile([P, GT * K], F32, tag="sf")
                        nc.vector.reduce_sum(
                            out=s_f[:, :gn * K],
                            in_=prod[:, :gn * d].rearrange("p (a e) -> p a e", e=dd),
                            axis=mybir.AxisListType.X,
                        )
                        e_f = sb1.tile([P, GT * K], F32, tag="ef")
                        nc.scalar.activation(
                            out=e_f[:, :gn * K], in_=s_f[:, :gn * K],
                            func=mybir.ActivationFunctionType.Exp,
                        )
                        q_f = sb1.tile([P, GT], F32, tag="qf")
                        nc.vector.reduce_sum(
                            out=q_f[:, :gn],
                            in_=e_f[:, :gn * K].rearrange("p (a k) -> p a k", k=K),
                            axis=mybir.AxisListType.X,
                        )
                        r_f = sb1.tile([P, GT], F32, tag="rf")
                        nc.vector.reciprocal(out=r_f[:, :gn], in_=q_f[:, :gn])
                        pe_f = sb1.tile([P, GT * K], BF16, tag="pef")
                        nc.vector.tensor_tensor(
                            out=pe_f[:, :gn * K].rearrange("p (a k) -> p a k", k=K),
                            in0=e_f[:, :gn * K].rearrange("p (a k) -> p a k", k=K),
                            in1=r_f[:, :gn].unsqueeze(2).to_broadcast([P, gn, K]),
                            op=mybir.AluOpType.mult,
                        )
                        msg = sb1.tile([P, GT * d], BF16, tag="msg")
                        nc.vector.tensor_tensor(
                            out=msg[:, :gn * d].rearrange("p (a e) -> p a e", e=dd),
                            in0=zch[:, g0 * d:(g0 + gn) * d].rearrange("p (a e) -> p a e", e=dd),
                            in1=pe_f[:, :gn * K].unsqueeze(2).to_broadcast([P, gn * K, dd]),
                            op=mybir.AluOpType.mult,
                        )
                        for t in range(gn):
                            nc.tensor.matmul(
                                out=seg_ps[:],
                                lhsT=S_sb[:, (g0 + t) * P:(g0 + t + 1) * P],
                                rhs=msg[:, t * d:(t + 1) * d],
                                start=(g0 + t == 0),
                                stop=(g0 + t == ntj - 1),
                            )
                    # chunk epilogue: u = l2norm(seg + hn)
                    tt = sc.tile([P, d], F32, tag="tt")
                    nc.vector.tensor_add(out=tt[:], in0=seg_ps[:], in1=hn[:, j * d:(j + 1) * d])
                    sq2 = sc.tile([P, d], F32, tag="sq2")
                    nc.vector.tensor_mul(out=sq2[:], in0=tt[:], in1=tt[:])
                    ss2 = sc.tile([P, K], F32, tag="ss2")
                    nc.vector.reduce_sum(
                        out=ss2[:], in_=sq2[:].rearrange("p (k e) -> p k e", k=K),
                        axis=mybir.AxisListType.X,
                    )
                    nr2 = sc.tile([P, K], F32, tag="nr2")
                    nc.scalar.activation(out=nr2[:], in_=ss2[:], func=mybir.ActivationFunctionType.Sqrt)
                    nc.vector.tensor_scalar_max(out=nr2[:], in0=nr2[:], scalar1=1e-12)
                    rr2 = sc.tile([P, K], F32, tag="rr2")
                    nc.vector.reciprocal(out=rr2[:], in_=nr2[:])
                    if it < NITER - 1:
                        nc.vector.tensor_tensor(
                            out=u_j[:].rearrange("p (k e) -> p k e", k=K),
                            in0=tt[:].rearrange("p (k e) -> p k e", k=K),
                            in1=rr2[:].unsqueeze(2).to_broadcast([P, K, dd]),
                            op=mybir.AluOpType.mult,
                        )
                    else:
                        # final: u (f32) -> lrelu -> clf matmul -> y
                        uf = sc.tile([P, d], F32, tag="uf")
                        nc.vector.tensor_tensor(
                            out=uf[:].rearrange("p (k e) -> p k e", k=K),
                            in0=tt[:].rearrange("p (k e) -> p k e", k=K),
                            in1=rr2[:].unsqueeze(2).to_broadcast([P, K, dd]),
                            op=mybir.AluOpType.mult,
                        )
                        us = sc.tile([P, d], F32, tag="us")
                        nc.vector.tensor_scalar_mul(out=us[:], in0=uf[:], scalar1=SLOPE)
                        nc.vector.tensor_tensor(out=uf[:], in0=uf[:], in1=us[:], op=mybir.AluOpType.max)
                        # transpose uf -> [d, nodes] K-tiles
                        uT_ps = psp.tile([P, 2 * P], F32, space="PSUM", tag="big")
                        nc.tensor.transpose(out=uT_ps[:, :P], in_=uf[:, :P], identity=idf[:])
                        nc.tensor.transpose(
                            out=uT_ps[: d - P, P:2 * P], in_=uf[:, P:d], identity=idf[:]
                        )
                        uT = sc.tile([P, 2 * P], F32, tag="uTs")
                        nc.vector.tensor_copy(out=uT[:, :P], in_=uT_ps[:, :P])
                        nc.vector.tensor_copy(out=uT[: d - P, P:], in_=uT_ps[: d - P, P:])
                        y_ps = psp.tile([P, nclass], F32, space="PSUM", tag="big")
                        nc.tensor.matmul(
                            out=y_ps[:], lhsT=uT[:, :P], rhs=cw_sb[:, :nclass],
                            start=True, stop=False,
                        )
                        nc.tensor.matmul(
                            out=y_ps[:], lhsT=uT[: d - P, P:], rhs=cw_sb[: d - P, nclass:],
                            start=False, stop=True,
                        )
                        y_sb = sc.tile([P, nclass], F32, tag="ysb")
                        nc.vector.tensor_add(out=y_sb[:], in0=y_ps[:], in1=cbrep[:])
                        nc.sync.dma_start(out=y_t[j * P:(j + 1) * P, :], in_=y_sb[:])
    return nc


_CACHE = {}
TRACE = False
LAST_RESULTS = None


def kernel(x, edge_index, pca_w, pca_b, clf_w, clf_b, n_cores=8, _sim=False):
    x = np.asarray(x, np.float32)
    edge_index = np.asarray(edge_index)
    idx_dtype = edge_index.dtype
    pca_w = np.asarray(pca_w, np.float32)
    pca_b = np.asarray(pca_b, np.float32)
    clf_w = np.asarray(clf_w, np.float32)
    clf_b = np.asarray(clf_b, np.float32)

    n, nfeat = x.shape
    d = pca_w.shape[1]
    nclass = clf_w.shape[1]

    nt, T, src_dev, lloc_dev, npc, nchunks = _host_prep(x, edge_index, n_cores)

    key = (n, nfeat, d, nclass, tuple(nt.tolist()))
    if key not in _CACHE:
        _CACHE[key] = build_program(nfeat, d, nclass, npc, nchunks, nt, T, n_cores, n)
        if not _sim:
            _split_multiwaits(_CACHE[key])
    nc = _CACHE[key]

    kf_pad = ((nfeat + P - 1) // P) * P
    npc_pad = nchunks * P
    w_pad = np.zeros((kf_pad, d), np.float32)
    w_pad[:nfeat] = pca_w
    brep = np.broadcast_to(pca_b, (P, d)).copy()
    cbrep = np.broadcast_to(clf_b, (P, nclass)).copy()

    in_maps = []
    for c in range(n_cores):
        xc = x[c * npc:(c + 1) * npc]
        xT = np.zeros((kf_pad, npc_pad), np.float32)
        xT[:nfeat, :npc] = xc.T
        in_maps.append({
            "xT": xT,
            "pca_w": w_pad,
            "pca_b_rep": brep,
            "clf_w": clf_w,
            "clf_b_rep": cbrep,
            "src": src_dev[c],
            "lloc": lloc_dev[c],
        })

    if _sim:
        from concourse.bass_interp import CoreSim
        assert n_cores == 1
        sim = CoreSim(nc)
        for kk, vv in in_maps[0].items():
            sim.tensor(kk)[:] = vv
        sim.simulate()
        return np.asarray(sim.tensor("y"))[:npc].astype(np.float32)
    global LAST_RESULTS
    res = run_bass_kernel_spmd(
        nc, in_maps, core_ids=list(range(n_cores)), trace=TRACE
    )
    LAST_RESULTS = res
    y = np.concatenate([res.results[c]["y"][:npc] for c in range(n_cores)], axis=0)
    return y.astype(np.float32)


if __name__ == "__main__":
    import pickle, time
    with open("/tmp/ref_inputs.pkl", "rb") as f:
        inputs = pickle.load(f)
    t0 = time.time()
    y = kernel(**inputs)
    print("kernel() wall time", time.time() - t0)
    np.save("/tmp/kernel_out.npy", y)


# revision 7
# speedup vs baseline: 1.2544x; 1.0245x over previous
"""DisentangledGNN Trainium2 kernel (8 NeuronCores, SPMD).

Strategy: target-bucketed node sharding. Each core owns n/8 consecutive
nodes and every edge whose target lands in that range. Per core:
  P0: pca matmul + leaky_relu + grouped l2norm for its own node slice
  P1: AllGather of the normalized features (bf16, padded to 192 cols)
  P2: one-time gather z = H[src] for its edges (indirect DMA)
  P3: 3 routing iterations, chunk-fused: for each 128-node chunk, edges
      are processed as 128-edge tiles; per-edge "gather u[trg]" and the
      segment-sum are one-hot matmuls on the tensor engine (S matrices
      built once per chunk via iota/is_equal); softmax over the 10
      factors is exp/sum on free axis (|s|<=1 so no max subtraction).
  P4: (fused in last iteration) leaky_relu + classifier matmul.
No inter-core communication during routing: a chunk's new u depends
only on that chunk's old u and the iteration-fixed z.
"""

import numpy as np
import ml_dtypes

import concourse.bass as bass
import concourse.mybir as mybir
import concourse.tile as tile
from concourse.masks import make_identity
from concourse.bass_utils import run_bass_kernel_spmd

F32 = mybir.dt.float32
BF16 = mybir.dt.bfloat16
I32 = mybir.dt.int32
I16 = mybir.dt.int16

K = 10
SLOPE = 0.01
NITER = 3
P = 128


def _split_multiwaits(nc):
    # This walrus accepts at most 1 sync wait per instruction (2 for
    # EventSemaphore ops); split extras onto preceding same-engine NOPs.
    n = [0]
    for fn in nc.m.functions:
        for blk in fn.blocks:
            newinsts = []
            changed = False
            for ins in blk.instructions:
                si = ins.sync_info
                cap = 2 if "EventSem" in type(ins).__name__ else 1
                if si is not None and len(si.on_wait) > cap:
                    waits = list(si.on_wait)
                    for w in waits[cap:]:
                        n[0] += 1
                        nop = mybir.InstNoOp(name=f"{ins.name}-ws{n[0]}", ins=[], outs=[])
                        nop.engine = ins.engine
                        nop.sync_info = mybir.SyncInfo(on_wait=[w], on_update=[])
                        newinsts.append(nop)
                    si.on_wait = waits[:cap]
                    ins.sync_info = si
                    changed = True
                newinsts.append(ins)
            if changed:
                blk.instructions = newinsts


def _host_prep(x, edge_index, n_cores):
    """Bucket edges by target core, chunk them by 128-node blocks,
    equalize per-chunk tile counts across cores, build per-core arrays."""
    n = x.shape[0]
    npc = n // n_cores            # nodes per core
    nchunks = (npc + P - 1) // P  # 128-node chunks per core
    src = np.asarray(edge_index[0], np.int64)
    trg = np.asarray(edge_index[1], np.int64)

    core_of = trg // npc
    ltrg = trg - core_of * npc

    # sort edges by (core, ltrg) once
    order = np.lexsort((ltrg, core_of))
    src_s, ltrg_s, core_s = src[order], ltrg[order], core_of[order]

    # per (core, chunk) counts
    chunk_s = ltrg_s // P
    counts = np.zeros((n_cores, nchunks), np.int64)
    np.add.at(counts, (core_s, chunk_s), 1)
    nt = np.maximum(1, (counts.max(axis=0) + P - 1) // P)  # tiles per chunk (shared)
    T = int(nt.sum())

    src_arr = np.zeros((n_cores, T * P), np.int32)
    lloc_arr = np.full((n_cores, T * P), 255, np.int16)  # 255 = dummy, never matches
    tile_of_chunk = np.concatenate([[0], np.cumsum(nt)]).astype(np.int64)

    core_starts = np.searchsorted(core_s, np.arange(n_cores + 1))
    for c in range(n_cores):
        cs, ce = core_starts[c], core_starts[c + 1]
        chunk_c = chunk_s[cs:ce]
        starts = np.searchsorted(chunk_c, np.arange(nchunks + 1))
        for j in range(nchunks):
            e0, e1 = cs + starts[j], cs + starts[j + 1]
            base = int(tile_of_chunk[j]) * P
            cnt = e1 - e0
            src_arr[c, base:base + cnt] = src_s[e0:e1]
            lloc_arr[c, base:base + cnt] = (ltrg_s[e0:e1] - j * P).astype(np.int16)

    # device wants [128, T] partition-major: edge t*128+p -> [p, t]
    src_dev = src_arr.reshape(n_cores, T, P).transpose(0, 2, 1).copy()
    lloc_dev = lloc_arr.reshape(n_cores, T, P).transpose(0, 2, 1).copy()
    return nt, T, src_dev, lloc_dev, npc, nchunks


def build_program(nfeat, d, nclass, npc, nchunks, nt, T, n_cores, n_nodes):
    dd = d // K
    kf = nfeat  # contraction for pca, padded to x128 on host
    kf_pad = ((nfeat + P - 1) // P) * P
    npc_pad = nchunks * P
    DPAD = d

    nc = bass.Bass(num_devices=n_cores)

    xT_t = nc.dram_tensor("xT", [kf_pad, npc_pad], F32, kind="ExternalInput")
    w_t = nc.dram_tensor("pca_w", [kf_pad, d], F32, kind="ExternalInput")
    brep_t = nc.dram_tensor("pca_b_rep", [P, d], F32, kind="ExternalInput")
    cw_t = nc.dram_tensor("clf_w", [d, nclass], F32, kind="ExternalInput")
    cbrep_t = nc.dram_tensor("clf_b_rep", [P, nclass], F32, kind="ExternalInput")
    src_t = nc.dram_tensor("src", [P, T], I32, kind="ExternalInput")
    lloc_t = nc.dram_tensor("lloc", [P, T], I16, kind="ExternalInput")
    y_t = nc.dram_tensor("y", [npc_pad, nclass], F32, kind="ExternalOutput")
    Hp = nc.dram_tensor("Hp", [n_nodes, DPAD], BF16, kind="Internal")

    with tile.TileContext(nc) as tc:
        with (
            tc.tile_pool(name="persist", bufs=1) as pp,
            tc.tile_pool(name="dram", bufs=1, space="DRAM") as dp,
            tc.tile_pool(name="sb", bufs=3) as sb,
            tc.tile_pool(name="sb1", bufs=2) as sb1,
            tc.tile_pool(name="schunk", bufs=3) as sc,
            tc.tile_pool(name="ps", bufs=3, space="PSUM") as psp,
            tc.tile_pool(name="psu", bufs=2, space="PSUM") as psu,
        ):
            # ---------------- constants / persistent state ----------------
            iota_rep = pp.tile([P, P], I16)
            nc.gpsimd.iota(iota_rep[:], pattern=[[1, P]], base=0, channel_multiplier=0)
            ident = pp.tile([P, P], BF16)
            make_identity(nc, ident[:])
            idf = pp.tile([P, P], F32)
            make_identity(nc, idf[:])

            nkt0 = kf_pad // P
            w_sb = pp.tile([P, nkt0 * d], F32)  # pca_w K-tiles side by side
            nc.sync.dma_start(
                out=w_sb[:].rearrange("p (a d) -> p a d", d=d),
                in_=w_t[:].rearrange("(a p) d -> p a d", p=P),
            )
            brep = pp.tile([P, d], F32)
            nc.sync.dma_start(out=brep[:], in_=brep_t[:])
            cw_sb = pp.tile([P, 2 * nclass], F32)  # clf_w K-tiles: [0:128], [128:160]
            nc.sync.dma_start(out=cw_sb[:, :nclass], in_=cw_t[:P, :])
            nc.sync.dma_start(out=cw_sb[: d - P, nclass:], in_=cw_t[P:, :])
            cbrep = pp.tile([P, nclass], F32)
            nc.sync.dma_start(out=cbrep[:], in_=cbrep_t[:])

            hn = pp.tile([P, nchunks * d], F32)   # normalized features, own nodes
            nc.vector.memset(hn[:], 0.0)

            # bounce buffers for allgather
            ag_in = dp.tile([npc, DPAD], BF16)

            # ---------------- P0: pca + lrelu + l2norm --------------------
            nkt = kf_pad // P
            for m in range(nchunks):
                xt = sb.tile([P, nkt * P], F32, tag="xt")
                nc.sync.dma_start(
                    out=xt[:].rearrange("p (a q) -> p a q", q=P),
                    in_=xT_t[:, m * P:(m + 1) * P].rearrange("(a p) q -> p a q", p=P),
                )
                h_ps = psp.tile([P, d], F32, space="PSUM", tag="big")
                for a in range(nkt):
                    nc.tensor.matmul(
                        out=h_ps[:],
                        lhsT=xt[:, a * P:(a + 1) * P],
                        rhs=w_sb[:, a * d:(a + 1) * d],
                        start=(a == 0),
                        stop=(a == nkt - 1),
                    )
                h = sb.tile([P, d], F32, tag="h_sb")
                nc.vector.tensor_add(out=h[:], in0=h_ps[:], in1=brep[:])
                hs = sb.tile([P, d], F32, tag="hs")
                nc.vector.tensor_scalar_mul(out=hs[:], in0=h[:], scalar1=SLOPE)
                nc.vector.tensor_tensor(out=h[:], in0=h[:], in1=hs[:], op=mybir.AluOpType.max)
                # grouped l2 norm
                sq = sb.tile([P, d], F32, tag="sq")
                nc.vector.tensor_mul(out=sq[:], in0=h[:], in1=h[:])
                ss = sb.tile([P, K], F32, tag="ss")
                nc.vector.reduce_sum(
                    out=ss[:], in_=sq[:].rearrange("p (k e) -> p k e", k=K),
                    axis=mybir.AxisListType.X,
                )
                nrm = sb.tile([P, K], F32, tag="nrm")
                nc.scalar.activation(out=nrm[:], in_=ss[:], func=mybir.ActivationFunctionType.Sqrt)
                nc.vector.tensor_scalar_max(out=nrm[:], in0=nrm[:], scalar1=1e-12)
                rr = sb.tile([P, K], F32, tag="rr")
                nc.vector.reciprocal(out=rr[:], in_=nrm[:])
                nc.vector.tensor_tensor(
                    out=hn[:, m * d:(m + 1) * d].rearrange("p (k e) -> p k e", k=K),
                    in0=h[:].rearrange("p (k e) -> p k e", k=K),
                    in1=rr[:].unsqueeze(2).to_broadcast([P, K, dd]),
                    op=mybir.AluOpType.mult,
                )
                # bf16 padded copy for allgather
                hb = sb.tile([P, DPAD], BF16, tag="hb")
                nc.vector.tensor_copy(out=hb[:], in_=hn[:, m * d:(m + 1) * d])
                rows = min(P, npc - m * P)
                nc.sync.dma_start(out=ag_in[m * P:m * P + rows, :], in_=hb[:rows, :])

            # ---------------- P1: allgather -------------------------------
            nc.gpsimd.collective_compute(
                "AllGather",
                mybir.AluOpType.bypass,
                replica_groups=[list(range(n_cores))],
                ins=[ag_in[:]],
                outs=[Hp.ap()],
            )

            # ---------------- P2: (z gathered per chunk in P3) -------------
            src_sb = pp.tile([P, T], I32)
            nc.sync.dma_start(out=src_sb[:], in_=src_t[:])

            # ---------------- P3: routing ---------------------------------
            lloc_all = pp.tile([P, T], I16)
            nc.sync.dma_start(out=lloc_all[:], in_=lloc_t[:])

            GT = 8  # tiles per DVE batch group
            for j in range(nchunks):
                t0, t1 = int(np.sum(nt[:j])), int(np.sum(nt[:j + 1]))
                ntj = t1 - t0
                # chunk-resident data
                zch = sc.tile([P, ntj * d], BF16, tag="zch")
                for t in range(ntj):
                    nc.gpsimd.indirect_dma_start(
                        out=zch[:, t * d:(t + 1) * d],
                        out_offset=None,
                        in_=Hp.ap(),
                        in_offset=bass.IndirectOffsetOnAxis(
                            ap=src_sb[:, t0 + t:t0 + t + 1], axis=0
                        ),
                    )
                S_sb = sc.tile([P, ntj * P], BF16, tag="S")
                ST_sb = sc.tile([P, ntj * P], BF16, tag="ST")
                for t in range(ntj):
                    nc.vector.tensor_tensor(
                        out=S_sb[:, t * P:(t + 1) * P],
                        in0=lloc_all[:, t0 + t:t0 + t + 1].to_broadcast([P, P]),
                        in1=iota_rep[:],
                        op=mybir.AluOpType.is_equal,
                    )
                # transposes in batches of 4 per PSUM tile
                for b0 in range(0, ntj, 4):
                    bn = min(4, ntj - b0)
                    tr_ps = psp.tile([P, 4 * P], BF16, space="PSUM", tag="big")
                    for t in range(bn):
                        nc.tensor.transpose(
                            out=tr_ps[:, t * P:(t + 1) * P],
                            in_=S_sb[:, (b0 + t) * P:(b0 + t + 1) * P],
                            identity=ident[:],
                        )
                    nc.scalar.copy(
                        out=ST_sb[:, b0 * P:(b0 + bn) * P], in_=tr_ps[:, :bn * P]
                    )

                u_j = sc.tile([P, d], BF16, tag="uj")
                nc.vector.tensor_copy(out=u_j[:], in_=hn[:, j * d:(j + 1) * d])

                for it in range(NITER):
                    seg_ps = psu.tile([P, d], F32, space="PSUM", tag="seg")
                    for g0 in range(0, ntj, GT):
                        gn = min(GT, ntj - g0)
                        ut_bf = sb1.tile([P, GT * d], BF16, tag="utbf")
                        for b0 in range(g0, g0 + gn, 3):
                            bn = min(3, g0 + gn - b0)
                            ut_ps = psp.tile([P, 3 * d], F32, space="PSUM", tag="big")
                            for t in range(bn):
                                nc.tensor.matmul(
                                    out=ut_ps[:, t * d:(t + 1) * d],
                                    lhsT=ST_sb[:, (b0 + t) * P:(b0 + t + 1) * P],
                                    rhs=u_j[:],
                                    start=True, stop=True,
                                )
                            nc.scalar.copy(
                                out=ut_bf[:, (b0 - g0) * d:(b0 - g0 + bn) * d],
                                in_=ut_ps[:, :bn * d],
                            )
                        prod = sb1.tile([P, GT * d], BF16, tag="prod")
                        nc.vector.tensor_mul(
                            out=prod[:, :gn * d],
                            in0=zch[:, g0 * d:(g0 + gn) * d],
                            in1=ut_bf[:, :gn * d],
                        )
                        t1 = sb1.tile([P, GT * d // 2], BF16, tag="t1")
                        nc.vector.tensor_add(
                            out=t1[:, :gn * d // 2].rearrange("p (a e) -> p a e", e=8),
                            in0=prod[:, :gn * d].rearrange("p (a e) -> p a e", e=dd)[:, :, 0:8],
                            in1=prod[:, :gn * d].rearrange("p (a e) -> p a e", e=dd)[:, :, 8:16],
                        )
                        t2 = sb1.tile([P, GT * d // 4], BF16, tag="t2")
                        nc.vector.tensor_add(
                            out=t2[:, :gn * d // 4].rearrange("p (a e) -> p a e", e=4),
                            in0=t1[:, :gn * d // 2].rearrange("p (a e) -> p a e", e=8)[:, :, 0:4],
                            in1=t1[:, :gn * d // 2].rearrange("p (a e) -> p a e", e=8)[:, :, 4:8],
                        )
                        t3 = sb1.tile([P, GT * d // 8], BF16, tag="t3")
                        nc.vector.tensor_add(
                            out=t3[:, :gn * d // 8].rearrange("p (a e) -> p a e", e=2),
                            in0=t2[:, :gn * d // 4].rearrange("p (a e) -> p a e", e=4)[:, :, 0:2],
                            in1=t2[:, :gn * d // 4].rearrange("p (a e) -> p a e", e=4)[:, :, 2:4],
                        )
                        s_f = sb1.tile([P, GT * K], F32, tag="sf")
                        nc.vector.tensor_add(
                            out=s_f[:, :gn * K],
                            in0=t3[:, :gn * d // 8].rearrange("p (a e) -> p a e", e=2)[:, :, 0:1].squeeze(2),
                            in1=t3[:, :gn * d // 8].rearrange("p (a e) -> p a e", e=2)[:, :, 1:2].squeeze(2),
                        )
                        e_f = sb1.tile([P, GT * K], F32, tag="ef")
                        nc.scalar.activation(
                            out=e_f[:, :gn * K], in_=s_f[:, :gn * K],
                            func=mybir.ActivationFunctionType.Exp,
                        )
                        q_f = sb1.tile([P, GT], F32, tag="qf")
                        nc.vector.reduce_sum(
                            out=q_f[:, :gn],
                            in_=e_f[:, :gn * K].rearrange("p (a k) -> p a k", k=K),
                            axis=mybir.AxisListType.X,
                        )
                        r_f = sb1.tile([P, GT], F32, tag="rf")
                        nc.vector.reciprocal(out=r_f[:, :gn], in_=q_f[:, :gn])
                        pe_f = sb1.tile([P, GT * K], BF16, tag="pef")
                        nc.vector.tensor_tensor(
                            out=pe_f[:, :gn * K].rearrange("p (a k) -> p a k", k=K),
                            in0=e_f[:, :gn * K].rearrange("p (a k) -> p a k", k=K),
                            in1=r_f[:, :gn].unsqueeze(2).to_broadcast([P, gn, K]),
                            op=mybir.AluOpType.mult,
                        )
                        msg = sb1.tile([P, GT * d], BF16, tag="msg")
                        nc.vector.tensor_tensor(
                            out=msg[:, :gn * d].rearrange("p (a e) -> p a e", e=dd),
                            in0=zch[:, g0 * d:(g0 + gn) * d].rearrange("p (a e) -> p a e", e=dd),
                            in1=pe_f[:, :gn * K].unsqueeze(2).to_broadcast([P, gn * K, dd]),
                            op=mybir.AluOpType.mult,
                        )
                        for t in range(gn):
                            nc.tensor.matmul(
                                out=seg_ps[:],
                                lhsT=S_sb[:, (g0 + t) * P:(g0 + t + 1) * P],
                                rhs=msg[:, t * d:(t + 1) * d],
                                start=(g0 + t == 0),
                                stop=(g0 + t == ntj - 1),
                            )
                    # chunk epilogue: u = l2norm(seg + hn)
                    tt = sc.tile([P, d], F32, tag="tt")
                    nc.vector.tensor_add(out=tt[:], in0=seg_ps[:], in1=hn[:, j * d:(j + 1) * d])
                    sq2 = sc.tile([P, d], F32, tag="sq2")
                    nc.vector.tensor_mul(out=sq2[:], in0=tt[:], in1=tt[:])
                    ss2 = sc.tile([P, K], F32, tag="ss2")
                    nc.vector.reduce_sum(
                        out=ss2[:], in_=sq2[:].rearrange("p (k e) -> p k e", k=K),
                        axis=mybir.AxisListType.X,
                    )
                    nr2 = sc.tile([P, K], F32, tag="nr2")
                    nc.scalar.activation(out=nr2[:], in_=ss2[:], func=mybir.ActivationFunctionType.Sqrt)
                    nc.vector.tensor_scalar_max(out=nr2[:], in0=nr2[:], scalar1=1e-12)
                    rr2 = sc.tile([P, K], F32, tag="rr2")
                    nc.vector.reciprocal(out=rr2[:], in_=nr2[:])
                    if it < NITER - 1:
                        nc.vector.tensor_tensor(
                            out=u_j[:].rearrange("p (k e) -> p k e", k=K),
                            in0=tt[:].rearrange("p (k e) -> p k e", k=K),
                            in1=rr2[:].unsqueeze(2).to_broadcast([P, K, dd]),
                            op=mybir.AluOpType.mult,
                        )
                    else:
                        # final: u (f32) -> lrelu -> clf matmul -> y
                        uf = sc.tile([P, d], F32, tag="uf")
                        nc.vector.tensor_tensor(
                            out=uf[:].rearrange("p (k e) -> p k e", k=K),
                            in0=tt[:].rearrange("p (k e) -> p k e", k=K),
                            in1=rr2[:].unsqueeze(2).to_broadcast([P, K, dd]),
                            op=mybir.AluOpType.mult,
                        )
                        us = sc.tile([P, d], F32, tag="us")
                        nc.vector.tensor_scalar_mul(out=us[:], in0=uf[:], scalar1=SLOPE)
                        nc.vector.tensor_tensor(out=uf[:], in0=uf[:], in1=us[:], op=mybir.AluOpType.max)
                        # transpose uf -> [d, nodes] K-tiles
                        uT_ps = psp.tile([P, 2 * P], F32, space="PSUM", tag="big")
                        nc.tensor.transpose(out=uT_ps[:, :P], in_=uf[:, :P], identity=idf[:])
                        nc.tensor.transpose(
                            out=uT_ps[: d - P, P:2 * P], in_=uf[:, P:d], identity=idf[:]
                        )
                        uT = sc.tile([P, 2 * P], F32, tag="uTs")
                        nc.vector.tensor_copy(out=uT[:, :P], in_=uT_ps[:, :P])
                        nc.vector.tensor_copy(out=uT[: d - P, P:], in_=uT_ps[: d - P, P:])
                        y_ps = psp.tile([P, nclass], F32, space="PSUM", tag="big")
                        nc.tensor.matmul(
                            out=y_ps[:], lhsT=uT[:, :P], rhs=cw_sb[:, :nclass],
                            start=True, stop=False,
                        )
                        nc.tensor.matmul(
                            out=y_ps[:], lhsT=uT[: d - P, P:], rhs=cw_sb[: d - P, nclass:],
                            start=False, stop=True,
                        )
                        y_sb = sc.tile([P, nclass], F32, tag="ysb")
                        nc.vector.tensor_add(out=y_sb[:], in0=y_ps[:], in1=cbrep[:])
                        nc.sync.dma_start(out=y_t[j * P:(j + 1) * P, :], in_=y_sb[:])
    return nc


_CACHE = {}
TRACE = False
LAST_RESULTS = None


def kernel(x, edge_index, pca_w, pca_b, clf_w, clf_b, n_cores=8, _sim=False):
    x = np.asarray(x, np.float32)
    edge_index = np.asarray(edge_index)
    idx_dtype = edge_index.dtype
    pca_w = np.asarray(pca_w, np.float32)
    pca_b = np.asarray(pca_b, np.float32)
    clf_w = np.asarray(clf_w, np.float32)
    clf_b = np.asarray(clf_b, np.float32)

    n, nfeat = x.shape
    d = pca_w.shape[1]
    nclass = clf_w.shape[1]

    nt, T, src_dev, lloc_dev, npc, nchunks = _host_prep(x, edge_index, n_cores)

    key = (n, nfeat, d, nclass, tuple(nt.tolist()))
    if key not in _CACHE:
        _CACHE[key] = build_program(nfeat, d, nclass, npc, nchunks, nt, T, n_cores, n)
        if not _sim:
            _split_multiwaits(_CACHE[key])
    nc = _CACHE[key]

    kf_pad = ((nfeat + P - 1) // P) * P
    npc_pad = nchunks * P
    w_pad = np.zeros((kf_pad, d), np.float32)
    w_pad[:nfeat] = pca_w
    brep = np.broadcast_to(pca_b, (P, d)).copy()
    cbrep = np.broadcast_to(clf_b, (P, nclass)).copy()

    in_maps = []
    for c in range(n_cores):
        xc = x[c * npc:(c + 1) * npc]
        xT = np.zeros((kf_pad, npc_pad), np.float32)
        xT[:nfeat, :npc] = xc.T
        in_maps.append({
            "xT": xT,
            "pca_w": w_pad,
            "pca_b_rep": brep,
            "clf_w": clf_w,
            "clf_b_rep": cbrep,
            "src": src_dev[c],
            "lloc": lloc_dev[c],
        })

    if _sim:
        from concourse.bass_interp import CoreSim
        assert n_cores == 1
        sim = CoreSim(nc)
        for kk, vv in in_maps[0].items():
            sim.tensor(kk)[:] = vv
        sim.simulate()
        return np.asarray(sim.tensor("y"))[:npc].astype(np.float32)
    global LAST_RESULTS
    res = run_bass_kernel_spmd(
        nc, in_maps, core_ids=list(range(n_cores)), trace=TRACE
    )
    LAST_RESULTS = res
    y = np.concatenate([res.results[c]["y"][:npc] for c in range(n_cores)], axis=0)
    return y.astype(np.float32)


if __name__ == "__main__":
    import pickle, time
    with open("/tmp/ref_inputs.pkl", "rb") as f:
        inputs = pickle.load(f)
    t0 = time.time()
    y = kernel(**inputs)
    print("kernel() wall time", time.time() - t0)
    np.save("/tmp/kernel_out.npy", y)


# revision 8
# speedup vs baseline: 1.5188x; 1.2107x over previous
"""DisentangledGNN Trainium2 kernel (8 NeuronCores, SPMD).

Strategy: target-bucketed node sharding. Each core owns n/8 consecutive
nodes and every edge whose target lands in that range. Per core:
  P0: pca matmul + leaky_relu + grouped l2norm for its own node slice
  P1: AllGather of the normalized features (bf16, padded to 192 cols)
  P2: one-time gather z = H[src] for its edges (indirect DMA)
  P3: 3 routing iterations, chunk-fused: for each 128-node chunk, edges
      are processed as 128-edge tiles; per-edge "gather u[trg]" and the
      segment-sum are one-hot matmuls on the tensor engine (S matrices
      built once per chunk via iota/is_equal); softmax over the 10
      factors is exp/sum on free axis (|s|<=1 so no max subtraction).
  P4: (fused in last iteration) leaky_relu + classifier matmul.
No inter-core communication during routing: a chunk's new u depends
only on that chunk's old u and the iteration-fixed z.
"""

import numpy as np
import ml_dtypes

import concourse.bass as bass
import concourse.mybir as mybir
import concourse.tile as tile
from concourse.masks import make_identity
from concourse.bass_utils import run_bass_kernel_spmd

F32 = mybir.dt.float32
BF16 = mybir.dt.bfloat16
I32 = mybir.dt.int32
I16 = mybir.dt.int16

K = 10
SLOPE = 0.01
NITER = 3
P = 128


def _split_multiwaits(nc):
    # This walrus accepts at most 1 sync wait per instruction (2 for
    # EventSemaphore ops); split extras onto preceding same-engine NOPs.
    n = [0]
    for fn in nc.m.functions:
        for blk in fn.blocks:
            newinsts = []
            changed = False
            for ins in blk.instructions:
                si = ins.sync_info
                cap = 2 if "EventSem" in type(ins).__name__ else 1
                if si is not None and len(si.on_wait) > cap:
                    waits = list(si.on_wait)
                    for w in waits[cap:]:
                        n[0] += 1
                        nop = mybir.InstNoOp(name=f"{ins.name}-ws{n[0]}", ins=[], outs=[])
                        nop.engine = ins.engine
                        nop.sync_info = mybir.SyncInfo(on_wait=[w], on_update=[])
                        newinsts.append(nop)
                    si.on_wait = waits[:cap]
                    ins.sync_info = si
                    changed = True
                newinsts.append(ins)
            if changed:
                blk.instructions = newinsts


def _host_prep(x, edge_index, n_cores):
    """Bucket edges by target core, chunk them by 128-node blocks,
    equalize per-chunk tile counts across cores, build per-core arrays."""
    n = x.shape[0]
    npc = n // n_cores            # nodes per core
    nchunks = (npc + P - 1) // P  # 128-node chunks per core
    src = np.asarray(edge_index[0], np.int64)
    trg = np.asarray(edge_index[1], np.int64)

    core_of = trg // npc
    ltrg = trg - core_of * npc

    # sort edges by (core, ltrg) once
    order = np.lexsort((ltrg, core_of))
    src_s, ltrg_s, core_s = src[order], ltrg[order], core_of[order]

    # per (core, chunk) counts
    chunk_s = ltrg_s // P
    counts = np.zeros((n_cores, nchunks), np.int64)
    np.add.at(counts, (core_s, chunk_s), 1)
    nt = np.maximum(1, (counts.max(axis=0) + P - 1) // P)  # tiles per chunk (shared)
    T = int(nt.sum())

    src_arr = np.zeros((n_cores, T * P), np.int32)
    lloc_arr = np.full((n_cores, T * P), 255, np.int16)  # 255 = dummy, never matches
    tile_of_chunk = np.concatenate([[0], np.cumsum(nt)]).astype(np.int64)

    core_starts = np.searchsorted(core_s, np.arange(n_cores + 1))
    for c in range(n_cores):
        cs, ce = core_starts[c], core_starts[c + 1]
        chunk_c = chunk_s[cs:ce]
        starts = np.searchsorted(chunk_c, np.arange(nchunks + 1))
        for j in range(nchunks):
            e0, e1 = cs + starts[j], cs + starts[j + 1]
            base = int(tile_of_chunk[j]) * P
            cnt = e1 - e0
            src_arr[c, base:base + cnt] = src_s[e0:e1]
            lloc_arr[c, base:base + cnt] = (ltrg_s[e0:e1] - j * P).astype(np.int16)

    # device wants [128, T] partition-major: edge t*128+p -> [p, t]
    src_dev = src_arr.reshape(n_cores, T, P).transpose(0, 2, 1).copy()
    lloc_dev = lloc_arr.reshape(n_cores, T, P).transpose(0, 2, 1).copy()
    return nt, T, src_dev, lloc_dev, npc, nchunks


def build_program(nfeat, d, nclass, npc, nchunks, nt, T, n_cores, n_nodes):
    dd = d // K
    kf = nfeat  # contraction for pca, padded to x128 on host
    kf_pad = ((nfeat + P - 1) // P) * P
    npc_pad = nchunks * P
    DPAD = d

    nc = bass.Bass(num_devices=n_cores)

    xT_t = nc.dram_tensor("xT", [kf_pad, npc_pad], F32, kind="ExternalInput")
    w_t = nc.dram_tensor("pca_w", [kf_pad, d], F32, kind="ExternalInput")
    brep_t = nc.dram_tensor("pca_b_rep", [P, d], F32, kind="ExternalInput")
    cw_t = nc.dram_tensor("clf_w", [d, nclass], F32, kind="ExternalInput")
    cbrep_t = nc.dram_tensor("clf_b_rep", [P, nclass], F32, kind="ExternalInput")
    src_t = nc.dram_tensor("src", [P, T], I32, kind="ExternalInput")
    lloc_t = nc.dram_tensor("lloc", [P, T], I16, kind="ExternalInput")
    y_t = nc.dram_tensor("y", [npc_pad, nclass], F32, kind="ExternalOutput")
    Hp = nc.dram_tensor("Hp", [n_nodes, DPAD], BF16, kind="Internal")

    with tile.TileContext(nc) as tc:
        with (
            tc.tile_pool(name="persist", bufs=1) as pp,
            tc.tile_pool(name="dram", bufs=1, space="DRAM") as dp,
            tc.tile_pool(name="sb", bufs=3) as sb,
            tc.tile_pool(name="sb1", bufs=2) as sb1,
            tc.tile_pool(name="schunk", bufs=3) as sc,
            tc.tile_pool(name="ps", bufs=3, space="PSUM") as psp,
            tc.tile_pool(name="psu", bufs=2, space="PSUM") as psu,
        ):
            # ---------------- constants / persistent state ----------------
            iota_rep = pp.tile([P, P], I16)
            nc.gpsimd.iota(iota_rep[:], pattern=[[1, P]], base=0, channel_multiplier=0)
            ident = pp.tile([P, P], BF16)
            make_identity(nc, ident[:])
            idf = pp.tile([P, P], F32)
            make_identity(nc, idf[:])

            nkt0 = kf_pad // P
            w_sb = pp.tile([P, nkt0 * d], F32)  # pca_w K-tiles side by side
            nc.sync.dma_start(
                out=w_sb[:].rearrange("p (a d) -> p a d", d=d),
                in_=w_t[:].rearrange("(a p) d -> p a d", p=P),
            )
            brep = pp.tile([P, d], F32)
            nc.sync.dma_start(out=brep[:], in_=brep_t[:])
            cw_sb = pp.tile([P, 2 * nclass], F32)  # clf_w K-tiles: [0:128], [128:160]
            nc.sync.dma_start(out=cw_sb[:, :nclass], in_=cw_t[:P, :])
            nc.sync.dma_start(out=cw_sb[: d - P, nclass:], in_=cw_t[P:, :])
            cbrep = pp.tile([P, nclass], F32)
            nc.sync.dma_start(out=cbrep[:], in_=cbrep_t[:])

            hn = pp.tile([P, nchunks * d], F32)   # normalized features, own nodes
            nc.vector.memset(hn[:], 0.0)

            # bounce buffers for allgather
            ag_in = dp.tile([npc, DPAD], BF16)

            # ---------------- P0: pca + lrelu + l2norm --------------------
            nkt = kf_pad // P
            for m in range(nchunks):
                xt = sb.tile([P, nkt * P], F32, tag="xt")
                nc.sync.dma_start(
                    out=xt[:].rearrange("p (a q) -> p a q", q=P),
                    in_=xT_t[:, m * P:(m + 1) * P].rearrange("(a p) q -> p a q", p=P),
                )
                h_ps = psp.tile([P, d], F32, space="PSUM", tag="big")
                for a in range(nkt):
                    nc.tensor.matmul(
                        out=h_ps[:],
                        lhsT=xt[:, a * P:(a + 1) * P],
                        rhs=w_sb[:, a * d:(a + 1) * d],
                        start=(a == 0),
                        stop=(a == nkt - 1),
                    )
                h = sb.tile([P, d], F32, tag="h_sb")
                nc.vector.tensor_add(out=h[:], in0=h_ps[:], in1=brep[:])
                hs = sb.tile([P, d], F32, tag="hs")
                nc.vector.tensor_scalar_mul(out=hs[:], in0=h[:], scalar1=SLOPE)
                nc.vector.tensor_tensor(out=h[:], in0=h[:], in1=hs[:], op=mybir.AluOpType.max)
                # grouped l2 norm
                sq = sb.tile([P, d], F32, tag="sq")
                nc.vector.tensor_mul(out=sq[:], in0=h[:], in1=h[:])
                ss = sb.tile([P, K], F32, tag="ss")
                nc.vector.reduce_sum(
                    out=ss[:], in_=sq[:].rearrange("p (k e) -> p k e", k=K),
                    axis=mybir.AxisListType.X,
                )
                nrm = sb.tile([P, K], F32, tag="nrm")
                nc.scalar.activation(out=nrm[:], in_=ss[:], func=mybir.ActivationFunctionType.Sqrt)
                nc.vector.tensor_scalar_max(out=nrm[:], in0=nrm[:], scalar1=1e-12)
                rr = sb.tile([P, K], F32, tag="rr")
                nc.vector.reciprocal(out=rr[:], in_=nrm[:])
                nc.vector.tensor_tensor(
                    out=hn[:, m * d:(m + 1) * d].rearrange("p (k e) -> p k e", k=K),
                    in0=h[:].rearrange("p (k e) -> p k e", k=K),
                    in1=rr[:].unsqueeze(2).to_broadcast([P, K, dd]),
                    op=mybir.AluOpType.mult,
                )
                # bf16 padded copy for allgather
                hb = sb.tile([P, DPAD], BF16, tag="hb")
                nc.vector.tensor_copy(out=hb[:], in_=hn[:, m * d:(m + 1) * d])
                rows = min(P, npc - m * P)
                nc.sync.dma_start(out=ag_in[m * P:m * P + rows, :], in_=hb[:rows, :])

            # ---------------- P1: allgather -------------------------------
            nc.gpsimd.collective_compute(
                "AllGather",
                mybir.AluOpType.bypass,
                replica_groups=[list(range(n_cores))],
                ins=[ag_in[:]],
                outs=[Hp.ap()],
            )

            # ---------------- P2: (z gathered per chunk in P3) -------------
            src_sb = pp.tile([P, T], I32)
            nc.sync.dma_start(out=src_sb[:], in_=src_t[:])

            # ---------------- P3: routing ---------------------------------
            lloc_all = pp.tile([P, T], I16)
            nc.sync.dma_start(out=lloc_all[:], in_=lloc_t[:])

            GT = 8  # tiles per DVE batch group

            def chunk_prologue(j):
                t0, t1 = int(np.sum(nt[:j])), int(np.sum(nt[:j + 1]))
                ntj = t1 - t0
                zch = sc.tile([P, ntj * d], BF16, tag=f"zch{j % 2}")
                for t in range(ntj):
                    nc.gpsimd.indirect_dma_start(
                        out=zch[:, t * d:(t + 1) * d],
                        out_offset=None,
                        in_=Hp.ap(),
                        in_offset=bass.IndirectOffsetOnAxis(
                            ap=src_sb[:, t0 + t:t0 + t + 1], axis=0
                        ),
                    )
                S_sb = sc.tile([P, ntj * P], BF16, tag=f"S{j % 2}")
                ST_sb = sc.tile([P, ntj * P], BF16, tag=f"ST{j % 2}")
                for t in range(ntj):
                    nc.vector.tensor_tensor(
                        out=S_sb[:, t * P:(t + 1) * P],
                        in0=lloc_all[:, t0 + t:t0 + t + 1].to_broadcast([P, P]),
                        in1=iota_rep[:],
                        op=mybir.AluOpType.is_equal,
                    )
                for b0 in range(0, ntj, 4):
                    bn = min(4, ntj - b0)
                    tr_ps = psp.tile([P, 4 * P], BF16, space="PSUM", tag="big")
                    for t in range(bn):
                        nc.tensor.transpose(
                            out=tr_ps[:, t * P:(t + 1) * P],
                            in_=S_sb[:, (b0 + t) * P:(b0 + t + 1) * P],
                            identity=ident[:],
                        )
                    nc.scalar.copy(
                        out=ST_sb[:, b0 * P:(b0 + bn) * P], in_=tr_ps[:, :bn * P]
                    )
                u_j = sc.tile([P, d], BF16, tag=f"uj{j % 2}")
                nc.vector.tensor_copy(out=u_j[:], in_=hn[:, j * d:(j + 1) * d])
                return dict(j=j, ntj=ntj, zch=zch, S_sb=S_sb, ST_sb=ST_sb, u_j=u_j)

            def chunk_groups(st):
                j, ntj, zch, S_sb, ST_sb, u_j = (
                    st["j"], st["ntj"], st["zch"], st["S_sb"], st["ST_sb"], st["u_j"]
                )
                seg_ps = psu.tile([P, d], F32, space="PSUM", tag=f"seg{j % 2}")
                st["seg_ps"] = seg_ps
                for g0 in range(0, ntj, GT):
                    gn = min(GT, ntj - g0)
                    ut_bf = sb1.tile([P, GT * d], BF16, tag="utbf")
                    for b0 in range(g0, g0 + gn, 3):
                        bn = min(3, g0 + gn - b0)
                        ut_ps = psp.tile([P, 3 * d], F32, space="PSUM", tag="big")
                        for t in range(bn):
                            nc.tensor.matmul(
                                out=ut_ps[:, t * d:(t + 1) * d],
                                lhsT=ST_sb[:, (b0 + t) * P:(b0 + t + 1) * P],
                                rhs=u_j[:],
                                start=True, stop=True,
                            )
                        nc.scalar.copy(
                            out=ut_bf[:, (b0 - g0) * d:(b0 - g0 + bn) * d],
                            in_=ut_ps[:, :bn * d],
                        )
                    prod = sb1.tile([P, GT * d], BF16, tag="prod")
                    nc.vector.tensor_mul(
                        out=prod[:, :gn * d],
                        in0=zch[:, g0 * d:(g0 + gn) * d],
                        in1=ut_bf[:, :gn * d],
                    )
                    t1_ = sb1.tile([P, GT * d // 2], BF16, tag="t1")
                    nc.vector.tensor_add(
                        out=t1_[:, :gn * d // 2].rearrange("p (a e) -> p a e", e=8),
                        in0=prod[:, :gn * d].rearrange("p (a e) -> p a e", e=dd)[:, :, 0:8],
                        in1=prod[:, :gn * d].rearrange("p (a e) -> p a e", e=dd)[:, :, 8:16],
                    )
                    t2_ = sb1.tile([P, GT * d // 4], BF16, tag="t2")
                    nc.vector.tensor_add(
                        out=t2_[:, :gn * d // 4].rearrange("p (a e) -> p a e", e=4),
                        in0=t1_[:, :gn * d // 2].rearrange("p (a e) -> p a e", e=8)[:, :, 0:4],
                        in1=t1_[:, :gn * d // 2].rearrange("p (a e) -> p a e", e=8)[:, :, 4:8],
                    )
                    t3_ = sb1.tile([P, GT * d // 8], BF16, tag="t3")
                    nc.vector.tensor_add(
                        out=t3_[:, :gn * d // 8].rearrange("p (a e) -> p a e", e=2),
                        in0=t2_[:, :gn * d // 4].rearrange("p (a e) -> p a e", e=4)[:, :, 0:2],
                        in1=t2_[:, :gn * d // 4].rearrange("p (a e) -> p a e", e=4)[:, :, 2:4],
                    )
                    s_f = sb1.tile([P, GT * K], F32, tag="sf")
                    nc.vector.tensor_add(
                        out=s_f[:, :gn * K],
                        in0=t3_[:, :gn * d // 8].rearrange("p (a e) -> p a e", e=2)[:, :, 0:1].squeeze(2),
                        in1=t3_[:, :gn * d // 8].rearrange("p (a e) -> p a e", e=2)[:, :, 1:2].squeeze(2),
                    )
                    e_f = sb1.tile([P, GT * K], F32, tag="ef")
                    nc.scalar.activation(
                        out=e_f[:, :gn * K], in_=s_f[:, :gn * K],
                        func=mybir.ActivationFunctionType.Exp,
                    )
                    q_f = sb1.tile([P, GT], F32, tag="qf")
                    nc.vector.reduce_sum(
                        out=q_f[:, :gn],
                        in_=e_f[:, :gn * K].rearrange("p (a k) -> p a k", k=K),
                        axis=mybir.AxisListType.X,
                    )
                    r_f = sb1.tile([P, GT], F32, tag="rf")
                    nc.vector.reciprocal(out=r_f[:, :gn], in_=q_f[:, :gn])
                    pe_f = sb1.tile([P, GT * K], BF16, tag="pef")
                    nc.vector.tensor_tensor(
                        out=pe_f[:, :gn * K].rearrange("p (a k) -> p a k", k=K),
                        in0=e_f[:, :gn * K].rearrange("p (a k) -> p a k", k=K),
                        in1=r_f[:, :gn].unsqueeze(2).to_broadcast([P, gn, K]),
                        op=mybir.AluOpType.mult,
                    )
                    msg = sb1.tile([P, GT * d], BF16, tag="msg")
                    nc.vector.tensor_tensor(
                        out=msg[:, :gn * d].rearrange("p (a e) -> p a e", e=dd),
                        in0=zch[:, g0 * d:(g0 + gn) * d].rearrange("p (a e) -> p a e", e=dd),
                        in1=pe_f[:, :gn * K].unsqueeze(2).to_broadcast([P, gn * K, dd]),
                        op=mybir.AluOpType.mult,
                    )
                    for t in range(gn):
                        nc.tensor.matmul(
                            out=seg_ps[:],
                            lhsT=S_sb[:, (g0 + t) * P:(g0 + t + 1) * P],
                            rhs=msg[:, t * d:(t + 1) * d],
                            start=(g0 + t == 0),
                            stop=(g0 + t == ntj - 1),
                        )

            def chunk_epilogue(st, it):
                j, u_j, seg_ps = st["j"], st["u_j"], st["seg_ps"]
                tt = sc.tile([P, d], F32, tag="tt")
                nc.vector.tensor_add(out=tt[:], in0=seg_ps[:], in1=hn[:, j * d:(j + 1) * d])
                sq2 = sc.tile([P, d], F32, tag="sq2")
                nc.vector.tensor_mul(out=sq2[:], in0=tt[:], in1=tt[:])
                ss2 = sc.tile([P, K], F32, tag="ss2")
                nc.vector.reduce_sum(
                    out=ss2[:], in_=sq2[:].rearrange("p (k e) -> p k e", k=K),
                    axis=mybir.AxisListType.X,
                )
                nr2 = sc.tile([P, K], F32, tag="nr2")
                nc.scalar.activation(out=nr2[:], in_=ss2[:], func=mybir.ActivationFunctionType.Sqrt)
                nc.vector.tensor_scalar_max(out=nr2[:], in0=nr2[:], scalar1=1e-12)
                rr2 = sc.tile([P, K], F32, tag="rr2")
                nc.vector.reciprocal(out=rr2[:], in_=nr2[:])
                if it < NITER - 1:
                    nc.vector.tensor_tensor(
                        out=u_j[:].rearrange("p (k e) -> p k e", k=K),
                        in0=tt[:].rearrange("p (k e) -> p k e", k=K),
                        in1=rr2[:].unsqueeze(2).to_broadcast([P, K, dd]),
                        op=mybir.AluOpType.mult,
                    )
                else:
                    uf = sc.tile([P, d], F32, tag="uf")
                    nc.vector.tensor_tensor(
                        out=uf[:].rearrange("p (k e) -> p k e", k=K),
                        in0=tt[:].rearrange("p (k e) -> p k e", k=K),
                        in1=rr2[:].unsqueeze(2).to_broadcast([P, K, dd]),
                        op=mybir.AluOpType.mult,
                    )
                    us = sc.tile([P, d], F32, tag="us")
                    nc.vector.tensor_scalar_mul(out=us[:], in0=uf[:], scalar1=SLOPE)
                    nc.vector.tensor_tensor(out=uf[:], in0=uf[:], in1=us[:], op=mybir.AluOpType.max)
                    uT_ps = psp.tile([P, 2 * P], F32, space="PSUM", tag="big")
                    nc.tensor.transpose(out=uT_ps[:, :P], in_=uf[:, :P], identity=idf[:])
                    nc.tensor.transpose(
                        out=uT_ps[: d - P, P:2 * P], in_=uf[:, P:d], identity=idf[:]
                    )
                    uT = sc.tile([P, 2 * P], F32, tag="uTs")
                    nc.vector.tensor_copy(out=uT[:, :P], in_=uT_ps[:, :P])
                    nc.vector.tensor_copy(out=uT[: d - P, P:], in_=uT_ps[: d - P, P:])
                    y_ps = psp.tile([P, nclass], F32, space="PSUM", tag="big")
                    nc.tensor.matmul(
                        out=y_ps[:], lhsT=uT[:, :P], rhs=cw_sb[:, :nclass],
                        start=True, stop=False,
                    )
                    nc.tensor.matmul(
                        out=y_ps[:], lhsT=uT[: d - P, P:], rhs=cw_sb[: d - P, nclass:],
                        start=False, stop=True,
                    )
                    y_sb = sc.tile([P, nclass], F32, tag="ysb")
                    nc.vector.tensor_add(out=y_sb[:], in0=y_ps[:], in1=cbrep[:])
                    nc.sync.dma_start(out=y_t[j * P:(j + 1) * P, :], in_=y_sb[:])

            # interleave chunk pairs so one chunk's epilogue chain hides
            # under the other's bulk work
            for j0 in range(0, nchunks, 2):
                stA = chunk_prologue(j0)
                stB = chunk_prologue(j0 + 1) if j0 + 1 < nchunks else None
                for it in range(NITER):
                    chunk_groups(stA)
                    if stB is not None:
                        chunk_groups(stB)
                    chunk_epilogue(stA, it)
                    if stB is not None:
                        chunk_epilogue(stB, it)
    return nc


_CACHE = {}
TRACE = False
LAST_RESULTS = None


def kernel(x, edge_index, pca_w, pca_b, clf_w, clf_b, n_cores=8, _sim=False):
    x = np.asarray(x, np.float32)
    edge_index = np.asarray(edge_index)
    idx_dtype = edge_index.dtype
    pca_w = np.asarray(pca_w, np.float32)
    pca_b = np.asarray(pca_b, np.float32)
    clf_w = np.asarray(clf_w, np.float32)
    clf_b = np.asarray(clf_b, np.float32)

    n, nfeat = x.shape
    d = pca_w.shape[1]
    nclass = clf_w.shape[1]

    nt, T, src_dev, lloc_dev, npc, nchunks = _host_prep(x, edge_index, n_cores)

    key = (n, nfeat, d, nclass, tuple(nt.tolist()))
    if key not in _CACHE:
        _CACHE[key] = build_program(nfeat, d, nclass, npc, nchunks, nt, T, n_cores, n)
        if not _sim:
            _split_multiwaits(_CACHE[key])
    nc = _CACHE[key]

    kf_pad = ((nfeat + P - 1) // P) * P
    npc_pad = nchunks * P
    w_pad = np.zeros((kf_pad, d), np.float32)
    w_pad[:nfeat] = pca_w
    brep = np.broadcast_to(pca_b, (P, d)).copy()
    cbrep = np.broadcast_to(clf_b, (P, nclass)).copy()

    in_maps = []
    for c in range(n_cores):
        xc = x[c * npc:(c + 1) * npc]
        xT = np.zeros((kf_pad, npc_pad), np.float32)
        xT[:nfeat, :npc] = xc.T
        in_maps.append({
            "xT": xT,
            "pca_w": w_pad,
            "pca_b_rep": brep,
            "clf_w": clf_w,
            "clf_b_rep": cbrep,
            "src": src_dev[c],
            "lloc": lloc_dev[c],
        })

    if _sim:
        from concourse.bass_interp import CoreSim
        assert n_cores == 1
        sim = CoreSim(nc)
        for kk, vv in in_maps[0].items():
            sim.tensor(kk)[:] = vv
        sim.simulate()
        return np.asarray(sim.tensor("y"))[:npc].astype(np.float32)
    global LAST_RESULTS
    res = run_bass_kernel_spmd(
        nc, in_maps, core_ids=list(range(n_cores)), trace=TRACE
    )
    LAST_RESULTS = res
    y = np.concatenate([res.results[c]["y"][:npc] for c in range(n_cores)], axis=0)
    return y.astype(np.float32)


if __name__ == "__main__":
    import pickle, time
    with open("/tmp/ref_inputs.pkl", "rb") as f:
        inputs = pickle.load(f)
    t0 = time.time()
    y = kernel(**inputs)
    print("kernel() wall time", time.time() - t0)
    np.save("/tmp/kernel_out.npy", y)


# revision 9
# speedup vs baseline: 1.5340x; 1.0101x over previous
"""DisentangledGNN Trainium2 kernel (8 NeuronCores, SPMD).

Strategy: target-bucketed node sharding. Each core owns n/8 consecutive
nodes and every edge whose target lands in that range. Per core:
  P0: pca matmul + leaky_relu + grouped l2norm for its own node slice
  P1: AllGather of the normalized features (bf16, padded to 192 cols)
  P2: one-time gather z = H[src] for its edges (indirect DMA)
  P3: 3 routing iterations, chunk-fused: for each 128-node chunk, edges
      are processed as 128-edge tiles; per-edge "gather u[trg]" and the
      segment-sum are one-hot matmuls on the tensor engine (S matrices
      built once per chunk via iota/is_equal); softmax over the 10
      factors is exp/sum on free axis (|s|<=1 so no max subtraction).
  P4: (fused in last iteration) leaky_relu + classifier matmul.
No inter-core communication during routing: a chunk's new u depends
only on that chunk's old u and the iteration-fixed z.
"""

import numpy as np
import ml_dtypes

import concourse.bass as bass
import concourse.mybir as mybir
import concourse.tile as tile
from concourse.masks import make_identity
from concourse.bass_utils import run_bass_kernel_spmd

F32 = mybir.dt.float32
BF16 = mybir.dt.bfloat16
I32 = mybir.dt.int32
I16 = mybir.dt.int16

K = 10
SLOPE = 0.01
NITER = 3
P = 128


def _split_multiwaits(nc):
    # This walrus accepts at most 1 sync wait per instruction (2 for
    # EventSemaphore ops); split extras onto preceding same-engine NOPs.
    n = [0]
    for fn in nc.m.functions:
        for blk in fn.blocks:
            newinsts = []
            changed = False
            for ins in blk.instructions:
                si = ins.sync_info
                cap = 2 if "EventSem" in type(ins).__name__ else 1
                if si is not None and len(si.on_wait) > cap:
                    waits = list(si.on_wait)
                    for w in waits[cap:]:
                        n[0] += 1
                        nop = mybir.InstNoOp(name=f"{ins.name}-ws{n[0]}", ins=[], outs=[])
                        nop.engine = ins.engine
                        nop.sync_info = mybir.SyncInfo(on_wait=[w], on_update=[])
                        newinsts.append(nop)
                    si.on_wait = waits[:cap]
                    ins.sync_info = si
                    changed = True
                newinsts.append(ins)
            if changed:
                blk.instructions = newinsts


def _host_prep(x, edge_index, n_cores):
    """Bucket edges by target core, chunk them by 128-node blocks,
    equalize per-chunk tile counts across cores, build per-core arrays."""
    n = x.shape[0]
    npc = n // n_cores            # nodes per core
    nchunks = (npc + P - 1) // P  # 128-node chunks per core
    src = np.asarray(edge_index[0], np.int64)
    trg = np.asarray(edge_index[1], np.int64)

    core_of = trg // npc
    ltrg = trg - core_of * npc

    # sort edges by (core, ltrg) once
    order = np.lexsort((ltrg, core_of))
    src_s, ltrg_s, core_s = src[order], ltrg[order], core_of[order]

    # per (core, chunk) counts
    chunk_s = ltrg_s // P
    counts = np.zeros((n_cores, nchunks), np.int64)
    np.add.at(counts, (core_s, chunk_s), 1)
    nt = np.maximum(1, (counts.max(axis=0) + P - 1) // P)  # tiles per chunk (shared)
    T = int(nt.sum())

    src_arr = np.zeros((n_cores, T * P), np.int32)
    lloc_arr = np.full((n_cores, T * P), 255, np.int16)  # 255 = dummy, never matches
    tile_of_chunk = np.concatenate([[0], np.cumsum(nt)]).astype(np.int64)

    core_starts = np.searchsorted(core_s, np.arange(n_cores + 1))
    for c in range(n_cores):
        cs, ce = core_starts[c], core_starts[c + 1]
        chunk_c = chunk_s[cs:ce]
        starts = np.searchsorted(chunk_c, np.arange(nchunks + 1))
        for j in range(nchunks):
            e0, e1 = cs + starts[j], cs + starts[j + 1]
            base = int(tile_of_chunk[j]) * P
            cnt = e1 - e0
            src_arr[c, base:base + cnt] = src_s[e0:e1]
            lloc_arr[c, base:base + cnt] = (ltrg_s[e0:e1] - j * P).astype(np.int16)

    # device wants [128, T] partition-major: edge t*128+p -> [p, t]
    src_dev = src_arr.reshape(n_cores, T, P).transpose(0, 2, 1).copy()
    lloc_dev = lloc_arr.reshape(n_cores, T, P).transpose(0, 2, 1).copy()
    return nt, T, src_dev, lloc_dev, npc, nchunks


def build_program(nfeat, d, nclass, npc, nchunks, nt, T, n_cores, n_nodes):
    dd = d // K
    kf = nfeat  # contraction for pca, padded to x128 on host
    kf_pad = ((nfeat + P - 1) // P) * P
    npc_pad = nchunks * P
    DPAD = d

    nc = bass.Bass(num_devices=n_cores)

    xT_t = nc.dram_tensor("xT", [kf_pad, npc_pad], F32, kind="ExternalInput")
    w_t = nc.dram_tensor("pca_w", [kf_pad, d], F32, kind="ExternalInput")
    brep_t = nc.dram_tensor("pca_b_rep", [P, d], F32, kind="ExternalInput")
    cw_t = nc.dram_tensor("clf_w", [d, nclass], F32, kind="ExternalInput")
    cbrep_t = nc.dram_tensor("clf_b_rep", [P, nclass], F32, kind="ExternalInput")
    src_t = nc.dram_tensor("src", [P, T], I32, kind="ExternalInput")
    lloc_t = nc.dram_tensor("lloc", [P, T], I16, kind="ExternalInput")
    y_t = nc.dram_tensor("y", [npc_pad, nclass], F32, kind="ExternalOutput")
    Hp = nc.dram_tensor("Hp", [n_nodes, DPAD], BF16, kind="Internal")

    with tile.TileContext(nc) as tc:
        with (
            tc.tile_pool(name="persist", bufs=1) as pp,
            tc.tile_pool(name="dram", bufs=1, space="DRAM") as dp,
            tc.tile_pool(name="sb", bufs=3) as sb,
            tc.tile_pool(name="sb1", bufs=2) as sb1,
            tc.tile_pool(name="schunk", bufs=3) as sc,
            tc.tile_pool(name="ps", bufs=3, space="PSUM") as psp,
            tc.tile_pool(name="psu", bufs=2, space="PSUM") as psu,
        ):
            # ---------------- constants / persistent state ----------------
            iota_rep = pp.tile([P, P], I16)
            nc.gpsimd.iota(iota_rep[:], pattern=[[1, P]], base=0, channel_multiplier=0)
            ident = pp.tile([P, P], BF16)
            make_identity(nc, ident[:])
            idf = pp.tile([P, P], F32)
            make_identity(nc, idf[:])

            nkt0 = kf_pad // P
            w_sb = pp.tile([P, nkt0 * d], F32)  # pca_w K-tiles side by side
            nc.sync.dma_start(
                out=w_sb[:].rearrange("p (a d) -> p a d", d=d),
                in_=w_t[:].rearrange("(a p) d -> p a d", p=P),
            )
            brep = pp.tile([P, d], F32)
            nc.sync.dma_start(out=brep[:], in_=brep_t[:])
            cw_sb = pp.tile([P, 2 * nclass], F32)  # clf_w K-tiles: [0:128], [128:160]
            nc.sync.dma_start(out=cw_sb[:, :nclass], in_=cw_t[:P, :])
            nc.sync.dma_start(out=cw_sb[: d - P, nclass:], in_=cw_t[P:, :])
            cbrep = pp.tile([P, nclass], F32)
            nc.sync.dma_start(out=cbrep[:], in_=cbrep_t[:])

            hn = pp.tile([P, nchunks * d], F32)   # normalized features, own nodes
            nc.vector.memset(hn[:], 0.0)

            # bounce buffers for allgather
            ag_in = dp.tile([npc, DPAD], BF16)

            # ---------------- P0: pca + lrelu + l2norm --------------------
            nkt = kf_pad // P
            for m in range(nchunks):
                xt = sb.tile([P, nkt * P], F32, tag="xt")
                nc.sync.dma_start(
                    out=xt[:].rearrange("p (a q) -> p a q", q=P),
                    in_=xT_t[:, m * P:(m + 1) * P].rearrange("(a p) q -> p a q", p=P),
                )
                h_ps = psp.tile([P, d], F32, space="PSUM", tag="big")
                for a in range(nkt):
                    nc.tensor.matmul(
                        out=h_ps[:],
                        lhsT=xt[:, a * P:(a + 1) * P],
                        rhs=w_sb[:, a * d:(a + 1) * d],
                        start=(a == 0),
                        stop=(a == nkt - 1),
                    )
                h = sb.tile([P, d], F32, tag="h_sb")
                nc.vector.tensor_add(out=h[:], in0=h_ps[:], in1=brep[:])
                hs = sb.tile([P, d], F32, tag="hs")
                nc.vector.tensor_scalar_mul(out=hs[:], in0=h[:], scalar1=SLOPE)
                nc.vector.tensor_tensor(out=h[:], in0=h[:], in1=hs[:], op=mybir.AluOpType.max)
                # grouped l2 norm
                sq = sb.tile([P, d], F32, tag="sq")
                nc.vector.tensor_mul(out=sq[:], in0=h[:], in1=h[:])
                ss = sb.tile([P, K], F32, tag="ss")
                nc.vector.reduce_sum(
                    out=ss[:], in_=sq[:].rearrange("p (k e) -> p k e", k=K),
                    axis=mybir.AxisListType.X,
                )
                nrm = sb.tile([P, K], F32, tag="nrm")
                nc.scalar.activation(out=nrm[:], in_=ss[:], func=mybir.ActivationFunctionType.Sqrt)
                nc.vector.tensor_scalar_max(out=nrm[:], in0=nrm[:], scalar1=1e-12)
                rr = sb.tile([P, K], F32, tag="rr")
                nc.vector.reciprocal(out=rr[:], in_=nrm[:])
                nc.vector.tensor_tensor(
                    out=hn[:, m * d:(m + 1) * d].rearrange("p (k e) -> p k e", k=K),
                    in0=h[:].rearrange("p (k e) -> p k e", k=K),
                    in1=rr[:].unsqueeze(2).to_broadcast([P, K, dd]),
                    op=mybir.AluOpType.mult,
                )
                # bf16 padded copy for allgather
                hb = sb.tile([P, DPAD], BF16, tag="hb")
                nc.vector.tensor_copy(out=hb[:], in_=hn[:, m * d:(m + 1) * d])
                rows = min(P, npc - m * P)
                nc.sync.dma_start(out=ag_in[m * P:m * P + rows, :], in_=hb[:rows, :])

            # ---------------- P1: allgather -------------------------------
            nc.gpsimd.collective_compute(
                "AllGather",
                mybir.AluOpType.bypass,
                replica_groups=[list(range(n_cores))],
                ins=[ag_in[:]],
                outs=[Hp.ap()],
            )

            # ---------------- P2: (z gathered per chunk in P3) -------------
            src_sb = pp.tile([P, T], I32)
            nc.sync.dma_start(out=src_sb[:], in_=src_t[:])

            # ---------------- P3: routing ---------------------------------
            lloc_all = pp.tile([P, T], I16)
            nc.sync.dma_start(out=lloc_all[:], in_=lloc_t[:])

            GT = 8  # tiles per DVE batch group

            def chunk_prologue(j):
                t0, t1 = int(np.sum(nt[:j])), int(np.sum(nt[:j + 1]))
                ntj = t1 - t0
                zch = sc.tile([P, ntj * d], BF16, tag=f"zch{j % 2}")
                for t in range(ntj):
                    nc.gpsimd.indirect_dma_start(
                        out=zch[:, t * d:(t + 1) * d],
                        out_offset=None,
                        in_=Hp.ap(),
                        in_offset=bass.IndirectOffsetOnAxis(
                            ap=src_sb[:, t0 + t:t0 + t + 1], axis=0
                        ),
                    )
                S_sb = sc.tile([P, ntj * P], BF16, tag=f"S{j % 2}")
                ST_sb = sc.tile([P, ntj * P], BF16, tag=f"ST{j % 2}")
                for t in range(ntj):
                    nc.vector.tensor_tensor(
                        out=S_sb[:, t * P:(t + 1) * P],
                        in0=lloc_all[:, t0 + t:t0 + t + 1].to_broadcast([P, P]),
                        in1=iota_rep[:],
                        op=mybir.AluOpType.is_equal,
                    )
                for b0 in range(0, ntj, 4):
                    bn = min(4, ntj - b0)
                    tr_ps = psp.tile([P, 4 * P], BF16, space="PSUM", tag="big")
                    for t in range(bn):
                        nc.tensor.transpose(
                            out=tr_ps[:, t * P:(t + 1) * P],
                            in_=S_sb[:, (b0 + t) * P:(b0 + t + 1) * P],
                            identity=ident[:],
                        )
                    nc.scalar.copy(
                        out=ST_sb[:, b0 * P:(b0 + bn) * P], in_=tr_ps[:, :bn * P]
                    )
                u_j = sc.tile([P, d], BF16, tag=f"uj{j % 2}")
                nc.vector.tensor_copy(out=u_j[:], in_=hn[:, j * d:(j + 1) * d])
                return dict(j=j, ntj=ntj, zch=zch, S_sb=S_sb, ST_sb=ST_sb, u_j=u_j)

            def chunk_groups(st):
                j, ntj, zch, S_sb, ST_sb, u_j = (
                    st["j"], st["ntj"], st["zch"], st["S_sb"], st["ST_sb"], st["u_j"]
                )
                seg_ps = psu.tile([P, d], F32, space="PSUM", tag=f"seg{j % 2}")
                st["seg_ps"] = seg_ps
                for g0 in range(0, ntj, GT):
                    gn = min(GT, ntj - g0)
                    ut_bf = sb1.tile([P, GT * d], BF16, tag="utbf")
                    for b0 in range(g0, g0 + gn, 3):
                        bn = min(3, g0 + gn - b0)
                        ut_ps = psp.tile([P, 3 * d], F32, space="PSUM", tag="big")
                        for t in range(bn):
                            nc.tensor.matmul(
                                out=ut_ps[:, t * d:(t + 1) * d],
                                lhsT=ST_sb[:, (b0 + t) * P:(b0 + t + 1) * P],
                                rhs=u_j[:],
                                start=True, stop=True,
                            )
                        nc.scalar.copy(
                            out=ut_bf[:, (b0 - g0) * d:(b0 - g0 + bn) * d],
                            in_=ut_ps[:, :bn * d],
                        )
                    prod = sb1.tile([P, GT * d], BF16, tag="prod")
                    nc.vector.tensor_mul(
                        out=prod[:, :gn * d],
                        in0=zch[:, g0 * d:(g0 + gn) * d],
                        in1=ut_bf[:, :gn * d],
                    )
                    t1_ = sb1.tile([P, GT * d // 2], BF16, tag="t1")
                    nc.vector.tensor_add(
                        out=t1_[:, :gn * d // 2].rearrange("p (a e) -> p a e", e=8),
                        in0=prod[:, :gn * d].rearrange("p (a e) -> p a e", e=dd)[:, :, 0:8],
                        in1=prod[:, :gn * d].rearrange("p (a e) -> p a e", e=dd)[:, :, 8:16],
                    )
                    t2_ = sb1.tile([P, GT * d // 4], BF16, tag="t2")
                    nc.vector.tensor_add(
                        out=t2_[:, :gn * d // 4].rearrange("p (a e) -> p a e", e=4),
                        in0=t1_[:, :gn * d // 2].rearrange("p (a e) -> p a e", e=8)[:, :, 0:4],
                        in1=t1_[:, :gn * d // 2].rearrange("p (a e) -> p a e", e=8)[:, :, 4:8],
                    )
                    t3_ = sb1.tile([P, GT * d // 8], BF16, tag="t3")
                    nc.vector.tensor_add(
                        out=t3_[:, :gn * d // 8].rearrange("p (a e) -> p a e", e=2),
                        in0=t2_[:, :gn * d // 4].rearrange("p (a e) -> p a e", e=4)[:, :, 0:2],
                        in1=t2_[:, :gn * d // 4].rearrange("p (a e) -> p a e", e=4)[:, :, 2:4],
                    )
                    s_f = sb1.tile([P, GT * K], F32, tag="sf")
                    nc.vector.tensor_add(
                        out=s_f[:, :gn * K],
                        in0=t3_[:, :gn * d // 8].rearrange("p (a e) -> p a e", e=2)[:, :, 0:1].squeeze(2),
                        in1=t3_[:, :gn * d // 8].rearrange("p (a e) -> p a e", e=2)[:, :, 1:2].squeeze(2),
                    )
                    e_f = sb1.tile([P, GT * K], F32, tag="ef")
                    nc.scalar.activation(
                        out=e_f[:, :gn * K], in_=s_f[:, :gn * K],
                        func=mybir.ActivationFunctionType.Exp,
                    )
                    q_f = sb1.tile([P, GT], F32, tag="qf")
                    nc.vector.reduce_sum(
                        out=q_f[:, :gn],
                        in_=e_f[:, :gn * K].rearrange("p (a k) -> p a k", k=K),
                        axis=mybir.AxisListType.X,
                    )
                    r_f = sb1.tile([P, GT], F32, tag="rf")
                    nc.vector.reciprocal(out=r_f[:, :gn], in_=q_f[:, :gn])
                    pe_f = sb1.tile([P, GT * K], BF16, tag="pef")
                    nc.vector.tensor_tensor(
                        out=pe_f[:, :gn * K].rearrange("p (a k) -> p a k", k=K),
                        in0=e_f[:, :gn * K].rearrange("p (a k) -> p a k", k=K),
                        in1=r_f[:, :gn].unsqueeze(2).to_broadcast([P, gn, K]),
                        op=mybir.AluOpType.mult,
                    )
                    pex = sb1.tile([P, GT * d], BF16, tag="pex")
                    nc.scalar.activation(
                        out=pex[:, :gn * d].rearrange("p (a e) -> p a e", e=dd),
                        in_=pe_f[:, :gn * K].unsqueeze(2).to_broadcast([P, gn * K, dd]),
                        func=mybir.ActivationFunctionType.Copy,
                    )
                    msg = sb1.tile([P, GT * d], BF16, tag="msg")
                    nc.vector.tensor_mul(
                        out=msg[:, :gn * d],
                        in0=zch[:, g0 * d:(g0 + gn) * d],
                        in1=pex[:, :gn * d],
                    )
                    for t in range(gn):
                        nc.tensor.matmul(
                            out=seg_ps[:],
                            lhsT=S_sb[:, (g0 + t) * P:(g0 + t + 1) * P],
                            rhs=msg[:, t * d:(t + 1) * d],
                            start=(g0 + t == 0),
                            stop=(g0 + t == ntj - 1),
                        )

            def chunk_epilogue(st, it):
                j, u_j, seg_ps = st["j"], st["u_j"], st["seg_ps"]
                tt = sc.tile([P, d], F32, tag="tt")
                nc.vector.tensor_add(out=tt[:], in0=seg_ps[:], in1=hn[:, j * d:(j + 1) * d])
                sq2 = sc.tile([P, d], F32, tag="sq2")
                nc.vector.tensor_mul(out=sq2[:], in0=tt[:], in1=tt[:])
                ss2 = sc.tile([P, K], F32, tag="ss2")
                nc.vector.reduce_sum(
                    out=ss2[:], in_=sq2[:].rearrange("p (k e) -> p k e", k=K),
                    axis=mybir.AxisListType.X,
                )
                nr2 = sc.tile([P, K], F32, tag="nr2")
                nc.scalar.activation(out=nr2[:], in_=ss2[:], func=mybir.ActivationFunctionType.Sqrt)
                nc.vector.tensor_scalar_max(out=nr2[:], in0=nr2[:], scalar1=1e-12)
                rr2 = sc.tile([P, K], F32, tag="rr2")
                nc.vector.reciprocal(out=rr2[:], in_=nr2[:])
                if it < NITER - 1:
                    nc.vector.tensor_tensor(
                        out=u_j[:].rearrange("p (k e) -> p k e", k=K),
                        in0=tt[:].rearrange("p (k e) -> p k e", k=K),
                        in1=rr2[:].unsqueeze(2).to_broadcast([P, K, dd]),
                        op=mybir.AluOpType.mult,
                    )
                else:
                    uf = sc.tile([P, d], F32, tag="uf")
                    nc.vector.tensor_tensor(
                        out=uf[:].rearrange("p (k e) -> p k e", k=K),
                        in0=tt[:].rearrange("p (k e) -> p k e", k=K),
                        in1=rr2[:].unsqueeze(2).to_broadcast([P, K, dd]),
                        op=mybir.AluOpType.mult,
                    )
                    us = sc.tile([P, d], F32, tag="us")
                    nc.vector.tensor_scalar_mul(out=us[:], in0=uf[:], scalar1=SLOPE)
                    nc.vector.tensor_tensor(out=uf[:], in0=uf[:], in1=us[:], op=mybir.AluOpType.max)
                    uT_ps = psp.tile([P, 2 * P], F32, space="PSUM", tag="big")
                    nc.tensor.transpose(out=uT_ps[:, :P], in_=uf[:, :P], identity=idf[:])
                    nc.tensor.transpose(
                        out=uT_ps[: d - P, P:2 * P], in_=uf[:, P:d], identity=idf[:]
                    )
                    uT = sc.tile([P, 2 * P], F32, tag="uTs")
                    nc.vector.tensor_copy(out=uT[:, :P], in_=uT_ps[:, :P])
                    nc.vector.tensor_copy(out=uT[: d - P, P:], in_=uT_ps[: d - P, P:])
                    y_ps = psp.tile([P, nclass], F32, space="PSUM", tag="big")
                    nc.tensor.matmul(
                        out=y_ps[:], lhsT=uT[:, :P], rhs=cw_sb[:, :nclass],
                        start=True, stop=False,
                    )
                    nc.tensor.matmul(
                        out=y_ps[:], lhsT=uT[: d - P, P:], rhs=cw_sb[: d - P, nclass:],
                        start=False, stop=True,
                    )
                    y_sb = sc.tile([P, nclass], F32, tag="ysb")
                    nc.vector.tensor_add(out=y_sb[:], in0=y_ps[:], in1=cbrep[:])
                    nc.sync.dma_start(out=y_t[j * P:(j + 1) * P, :], in_=y_sb[:])

            # interleave chunk pairs so one chunk's epilogue chain hides
            # under the other's bulk work
            for j0 in range(0, nchunks, 2):
                stA = chunk_prologue(j0)
                stB = chunk_prologue(j0 + 1) if j0 + 1 < nchunks else None
                for it in range(NITER):
                    chunk_groups(stA)
                    if stB is not None:
                        chunk_groups(stB)
                    chunk_epilogue(stA, it)
                    if stB is not None:
                        chunk_epilogue(stB, it)
    return nc


_CACHE = {}
TRACE = False
LAST_RESULTS = None


def kernel(x, edge_index, pca_w, pca_b, clf_w, clf_b, n_cores=8, _sim=False):
    x = np.asarray(x, np.float32)
    edge_index = np.asarray(edge_index)
    idx_dtype = edge_index.dtype
    pca_w = np.asarray(pca_w, np.float32)
    pca_b = np.asarray(pca_b, np.float32)
    clf_w = np.asarray(clf_w, np.float32)
    clf_b = np.asarray(clf_b, np.float32)

    n, nfeat = x.shape
    d = pca_w.shape[1]
    nclass = clf_w.shape[1]

    nt, T, src_dev, lloc_dev, npc, nchunks = _host_prep(x, edge_index, n_cores)

    key = (n, nfeat, d, nclass, tuple(nt.tolist()))
    if key not in _CACHE:
        _CACHE[key] = build_program(nfeat, d, nclass, npc, nchunks, nt, T, n_cores, n)
        if not _sim:
            _split_multiwaits(_CACHE[key])
    nc = _CACHE[key]

    kf_pad = ((nfeat + P - 1) // P) * P
    npc_pad = nchunks * P
    w_pad = np.zeros((kf_pad, d), np.float32)
    w_pad[:nfeat] = pca_w
    brep = np.broadcast_to(pca_b, (P, d)).copy()
    cbrep = np.broadcast_to(clf_b, (P, nclass)).copy()

    in_maps = []
    for c in range(n_cores):
        xc = x[c * npc:(c + 1) * npc]
        xT = np.zeros((kf_pad, npc_pad), np.float32)
        xT[:nfeat, :npc] = xc.T
        in_maps.append({
            "xT": xT,
            "pca_w": w_pad,
            "pca_b_rep": brep,
            "clf_w": clf_w,
            "clf_b_rep": cbrep,
            "src": src_dev[c],
            "lloc": lloc_dev[c],
        })

    if _sim:
        from concourse.bass_interp import CoreSim
        assert n_cores == 1
        sim = CoreSim(nc)
        for kk, vv in in_maps[0].items():
            sim.tensor(kk)[:] = vv
        sim.simulate()
        return np.asarray(sim.tensor("y"))[:npc].astype(np.float32)
    global LAST_RESULTS
    res = run_bass_kernel_spmd(
        nc, in_maps, core_ids=list(range(n_cores)), trace=TRACE
    )
    LAST_RESULTS = res
    y = np.concatenate([res.results[c]["y"][:npc] for c in range(n_cores)], axis=0)
    return y.astype(np.float32)


if __name__ == "__main__":
    import pickle, time
    with open("/tmp/ref_inputs.pkl", "rb") as f:
        inputs = pickle.load(f)
    t0 = time.time()
    y = kernel(**inputs)
    print("kernel() wall time", time.time() - t0)
    np.save("/tmp/kernel_out.npy", y)
